# revision 1
# baseline (speedup 1.0000x reference)
"""Trainium2 Bass kernel for a dense transformer encoder layer — v2.1.

Sharding: tensor-parallel attention within each batch group of 4 cores.
Core (b=core//4, r=core%4) computes Q/K/V and attention for heads
4r..4r+3 over ALL 2048 rows of batch b, multiplies by its 256-row slice
of W_O (per-core weight content), and a pipelined 4-core ReduceScatter
(bf16) per q-quarter sums the partial attention outputs, delivering
each core its own row-tiles rank-independently (own rows of core (b, r)
are the 4 tiles {j*512 + r*128 + [0,128)}).  LN1/residual, FFN and LN2
then run row-parallel.  A dummy warmup collective at kernel start
absorbs the ~130us one-time collective-channel initialization.

Attention inner loop: the two head-pairs of a dk-tile are interleaved
(scores p0, scores p1, ctx p0 (prev), ctx p1 (prev)) so every
dependency (exp, psum reuse) is >=1 slot away and the PE stream stays
dense enough to hold its boost clock.  ctx psums are drained eagerly
(unnormalized ctx + denominator row to SBUF) so the 2-bank ctx psum
rotation never waits on the slow DVE reciprocal; normalization (1/den
ones-matmul broadcast, in-place multiply) runs two pairs behind.  W_O
partials for quarter n are emitted during quarter n+1 so they never
wait on normalization.  FFN1 runs in two moving-column passes (q-tiles
0..2, then 3) so the last ReduceScatter + LN1 hide under ~40us of
FFN1-A matmuls.  The scalar engine is pinned to the natural_log_exp
table once (exp for attention, exp(-0.5*ln(var/D+eps)) for LN rstd,
relu for FFN1), avoiding all 1.28us table reloads.
"""

import contextlib

import numpy as np

import concourse.bass as bass
import concourse.mybir as mybir
from concourse import bacc
from concourse import bass_utils
from concourse.masks import make_identity
from concourse.tile import TileContext

F32 = mybir.dt.float32
F32R = mybir.dt.float32r
BF16 = mybir.dt.bfloat16
AF = mybir.ActivationFunctionType
OP = mybir.AluOpType

B, S, D, H, DKH, DFF = 2, 2048, 1024, 16, 64, 4096
EPS = 1e-6
NCORES = 8
GROUPS = [[0, 1, 2, 3], [4, 5, 6, 7]]
NDC = D // 128          # 8 contraction chunks
NKT = S // 128          # 16 key tiles
NQ = S // 512           # 4 q-quarters
NFT = DFF // 128        # 32 FFN tiles
VP = DKH + 1            # 65
LN_EXP_TABLE = 6        # natural_log_exp_and_others in act_info.json


def _build():
    nc = bacc.Bacc(None, num_devices=NCORES)

    xt = nc.declare_dram_parameter("xt", [D, S], BF16, isOutput=False)
    wqh = nc.declare_dram_parameter("wqh", [2, 128, D], BF16, isOutput=False)
    wkh = nc.declare_dram_parameter("wkh", [2, 128, D], BF16, isOutput=False)
    wvh = nc.declare_dram_parameter("wvh", [D, 256], BF16, isOutput=False)
    woh = nc.declare_dram_parameter("woh", [2, 128, D], BF16, isOutput=False)
    w1b = nc.declare_dram_parameter("w1b", [NFT, 128, D], BF16, isOutput=False)
    w2 = nc.declare_dram_parameter("w2", [DFF, D], BF16, isOutput=False)
    b1m = nc.declare_dram_parameter("b1m", [128, NFT], F32, isOutput=False)
    gb1 = nc.declare_dram_parameter("gb1", [128, D], BF16, isOutput=False)
    gb2 = nc.declare_dram_parameter("gb2", [128, D], BF16, isOutput=False)
    bb2 = nc.declare_dram_parameter("bb2", [128, D], F32, isOutput=False)
    b2b = nc.declare_dram_parameter("b2b", [128, D], F32, isOutput=False)
    keep = nc.declare_dram_parameter("keep", [128, NQ], F32, isOutput=False)
    xqb = nc.declare_dram_parameter("xqb", [512, D], F32, isOutput=False)
    out = nc.declare_dram_parameter("out", [512, D], F32, isOutput=True)

    with TileContext(nc) as tc:
        with tc.tile_pool(name="constp", bufs=1) as constp, \
             tc.tile_pool(name="dramp", bufs=1, space="DRAM") as dramp, \
             tc.tile_pool(name="persist", bufs=1) as persist:
            # pin the act table once: exp/ln/relu/copy all live in it
            nc.scalar.add_instruction(mybir.InstLoadActFuncSet(
                name=f"I-{nc.next_id()}", ins=[], outs=[],
                act_func_set_id=LN_EXP_TABLE))
            epsb = constp.tile([128, 1], F32, name="epsb")
            nc.vector.memset(epsb[:], EPS)
            ones_f = constp.tile([128, 128], F32, name="ones_f")
            nc.vector.memset(ones_f[:], 1.0)
            ones64 = constp.tile([128, 128], F32R, name="ones64")
            nc.vector.tensor_copy(ones64[:], ones_f[:])

            # warmup collective
            wz = constp.tile([128, 8], F32, name="wz")
            nc.vector.memset(wz[:], 0.0)
            biw = dramp.tile([4, 128, 8], F32, name="biw")
            bow = dramp.tile([128, 8], F32, name="bow")
            for g in range(4):
                nc.sync.dma_start(out=biw[g], in_=wz[:])
            nc.gpsimd.collective_compute(
                "ReduceScatter", OP.add, replica_groups=GROUPS,
                ins=[biw[:].opt()], outs=[bow[:].opt()])

            bi = [dramp.tile([4, 128, D], BF16, name=f"bi{j}") for j in range(NQ)]
            bo = [dramp.tile([128, D], BF16, name=f"bo{j}") for j in range(NQ)]

            x1 = [persist.tile([128, D], F32, name=f"x1_{i}", tag=f"x1_{i}")
                  for i in range(NQ)]

            with tc.tile_pool(name="lncp", bufs=1) as lncp, \
                 tc.tile_pool(name="lnp", bufs=1) as lnp, \
                 tc.tile_pool(name="expp", bufs=4) as expp, \
                 tc.tile_pool(name="wostg", bufs=3) as wostg:
                attns_st = contextlib.ExitStack()
                attns = attns_st.enter_context(tc.tile_pool(name="attns", bufs=1))
                q_sb = [attns.tile([128, S], BF16, name=f"q{t}", tag=f"q{t}")
                        for t in range(2)]
                k_sb = [attns.tile([128, S], BF16, name=f"k{t}", tag=f"k{t}")
                        for t in range(2)]
                v_sb = [attns.tile([128, 4 * VP], BF16, name=f"v{t}", tag=f"v{t}")
                        for t in range(NKT)]
                ctxa = [attns.tile([128, S], BF16, name=f"ca{t}", tag=f"ca{t}")
                        for t in range(2)]
                wo_sb = [attns.tile([128, D], BF16, name=f"wo{t}", tag=f"wo{t}")
                         for t in range(2)]

                # ---- projections ----
                with tc.tile_pool(name="xthp", bufs=1) as xthp, \
                     tc.tile_pool(name="wproj", bufs=1) as wproj, \
                     tc.tile_pool(name="psP", bufs=3, space="PSUM") as psP:
                    wq_sb = [wproj.tile([128, D], BF16, name=f"wq{t}", tag=f"wq{t}")
                             for t in range(2)]
                    wk_sb = [wproj.tile([128, D], BF16, name=f"wk{t}", tag=f"wk{t}")
                             for t in range(2)]
                    wvc = [wproj.tile([128, 256], BF16, name=f"wv{c}", tag=f"wv{c}")
                           for c in range(NDC)]
                    for t2 in range(2):
                        nc.sync.dma_start(out=wq_sb[t2][:], in_=wqh[t2])
                        nc.sync.dma_start(out=wk_sb[t2][:], in_=wkh[t2])
                    for c in range(NDC):
                        nc.sync.dma_start(out=wvc[c][:],
                                          in_=wvh[c * 128:(c + 1) * 128, :])
                    for t2 in range(2):
                        nc.sync.dma_start(out=wo_sb[t2][:], in_=woh[t2])
                    xth = [xthp.tile([128, S], BF16, name=f"xth{c}", tag=f"xth{c}")
                           for c in range(NDC)]
                    for c in range(NDC):
                        nc.sync.dma_start(out=xth[c][:],
                                          in_=xt[c * 128:(c + 1) * 128, :])

                    for t2 in range(2):
                        for n in range(NQ):
                            for w_sb, dst in ((wq_sb, q_sb), (wk_sb, k_sb)):
                                ps = psP.tile([128, 512], F32, name="psp",
                                              tag="psp")
                                for c in range(NDC):
                                    nc.tensor.matmul(
                                        ps[:], w_sb[t2][:, c * 128:(c + 1) * 128],
                                        xth[c][:, n * 512:(n + 1) * 512],
                                        start=(c == 0), stop=(c == NDC - 1))
                                nc.vector.tensor_copy(
                                    dst[t2][:, n * 512:(n + 1) * 512], ps[:])

                    for kt in range(NKT):
                        vr = v_sb[kt][:].rearrange("p (h c) -> p h c", c=VP)
                        nc.vector.tensor_copy(vr[:, :, DKH], ones_f[:, 0:4])
                    for kt in range(NKT):
                        ps = psP.tile([128, 256], F32, name="psv", tag="psv")
                        for c in range(NDC):
                            nc.tensor.matmul(
                                ps[:], xth[c][:, kt * 128:(kt + 1) * 128],
                                wvc[c][:],
                                start=(c == 0), stop=(c == NDC - 1))
                        vr = v_sb[kt][:].rearrange("p (h c) -> p h c", c=VP)
                        nc.vector.tensor_copy(
                            vr[:, :, 0:DKH],
                            ps[:].rearrange("p (h c) -> p h c", c=DKH))

                # ---- attention + lazy W_O + pipelined ReduceScatter ----
                if True:
                    att_ps = contextlib.ExitStack()
                    psS = att_ps.enter_context(
                        tc.tile_pool(name="psS", bufs=2, space="PSUM"))
                    psC = att_ps.enter_context(
                        tc.tile_pool(name="psC", bufs=2, space="PSUM"))
                    psW = att_ps.enter_context(
                        tc.tile_pool(name="psW", bufs=2, space="PSUM"))
                    keep_sb = lncp.tile([128, NQ], F32, name="keep_sb")
                    nc.sync.dma_start(out=keep_sb[:], in_=keep[:, :])
                    gb1_sb = lncp.tile([128, D], BF16, name="gb1_sb")
                    nc.sync.dma_start(out=gb1_sb[:], in_=gb1[:, :])
                    xqb_sb = [lncp.tile([128, D], F32, name=f"xqb{i}", tag=f"xqb{i}")
                              for i in range(NQ)]
                    for i in range(NQ):
                        nc.sync.dma_start(out=xqb_sb[i][:],
                                          in_=xqb[i * 128:(i + 1) * 128, :])

                    def emit_norm(p):
                        t2_, h2_, n_, rcp = p
                        rb = psW.tile([128, 512], F32, name="psw", tag="psw")
                        nc.tensor.matmul(rb[:, :], ones64[0:1, :],
                                         rcp[0:1, :], start=True, stop=True)
                        rbs = expp.tile([128, 512], BF16, name="rbs", tag="rbs",
                                        bufs=2)
                        nc.vector.tensor_copy(
                            rbs[h2_ * 64:(h2_ + 1) * 64, :],
                            rb[h2_ * 64:(h2_ + 1) * 64, :])
                        sl = ctxa[t2_][h2_ * 64:(h2_ + 1) * 64,
                                       n_ * 512:(n_ + 1) * 512]
                        nc.vector.tensor_mul(sl, sl,
                                             rbs[h2_ * 64:(h2_ + 1) * 64, :])

                    def emit_wo_rs(n):
                        for p in range(4):
                            qi = 4 * n + p
                            stg = wostg.tile([128, D], BF16, name="stg",
                                             tag="stg")
                            for n2 in range(2):
                                ps = psW.tile([128, 512], F32, name="psw",
                                              tag="psw")
                                for t2_ in range(2):
                                    nc.tensor.matmul(
                                        ps[:],
                                        ctxa[t2_][:, qi * 128:(qi + 1) * 128],
                                        wo_sb[t2_][:, n2 * 512:(n2 + 1) * 512],
                                        start=(t2_ == 0), stop=(t2_ == 1))
                                nc.vector.tensor_copy(
                                    stg[:, n2 * 512:(n2 + 1) * 512], ps[:])
                            nc.sync.dma_start(out=bi[n][p], in_=stg[:])
                        nc.gpsimd.collective_compute(
                            "ReduceScatter", OP.add, replica_groups=GROUPS,
                            ins=[bi[n][:].opt()], outs=[bo[n][:].opt()])

                    def emit_ln1(j):
                        ao = lnp.tile([128, D], BF16, name="ao", tag="ao")
                        nc.sync.dma_start(out=ao[:], in_=bo[j][:])
                        aom = lnp.tile([128, D], BF16, name="aom", tag="aom")
                        nc.vector.tensor_scalar_mul(aom[:], ao[:],
                                                    keep_sb[:, j:j + 1])
                        stat = lnp.tile([128, 4], F32, name="stat", tag="stat")
                        cent = lnp.tile([128, D], BF16, name="cent", tag="cent")
                        sq = lnp.tile([128, D], BF16, name="sq", tag="sq")
                        nc.vector.tensor_reduce(stat[:, 0:1], aom[:],
                                                mybir.AxisListType.X, OP.add)
                        nc.vector.tensor_scalar_mul(stat[:, 1:2], stat[:, 0:1],
                                                    1.0 / D)
                        nc.vector.tensor_scalar_sub(cent[:], aom[:],
                                                    stat[:, 1:2])
                        nc.vector.scalar_tensor_tensor(
                            sq[:], aom[:], stat[:, 1:2], cent[:],
                            op0=OP.subtract, op1=OP.mult, accum_out=stat[:, 2:3])
                        nc.scalar.activation(stat[:, 3:4], stat[:, 2:3], AF.Ln,
                                             bias=epsb[:, 0:1], scale=1.0 / D)
                        nc.scalar.activation(stat[:, 0:1], stat[:, 3:4], AF.Exp,
                                             scale=-0.5)
                        t1 = lnp.tile([128, D], F32, name="t1", tag="t1")
                        nc.vector.scalar_tensor_tensor(
                            t1[:], cent[:], stat[:, 0:1], gb1_sb[:],
                            op0=OP.mult, op1=OP.mult)
                        nc.vector.tensor_add(x1[j][:], t1[:], xqb_sb[j][:])

                    norm_q = []
                    pending_wo = None   # quarter whose W_O is not yet emitted
                    pending_ln = []     # quarters whose LN1 is not yet emitted
                    for n in range(NQ):
                        for t2 in range(2):
                            cps = {h2: psC.tile([VP, 512], F32, name="cps",
                                                tag="cps") for h2 in (0, 1)}
                            prev = None

                            def ctx_block(pk, pexs, stop):
                                for h2 in (0, 1):
                                    h = 2 * t2 + h2
                                    for u in (0, 1):
                                        kt = 2 * pk + u
                                        nc.tensor.matmul(
                                            cps[h2][:],
                                            v_sb[kt][:, h * VP:(h + 1) * VP],
                                            pexs[h2][:, u * 512:(u + 1) * 512],
                                            start=(pk == 0 and u == 0),
                                            stop=(stop and u == 1))

                            for kt2 in range(NKT // 2):
                                exs = {}
                                for h2 in (0, 1):
                                    sps = psS.tile([128, 1024], F32, name="sps",
                                                   tag="sps")
                                    for u in (0, 1):
                                        kt = 2 * kt2 + u
                                        nc.tensor.matmul(
                                            sps[:, u * 512:(u + 1) * 512],
                                            k_sb[t2][h2 * 64:(h2 + 1) * 64,
                                                     kt * 128:(kt + 1) * 128],
                                            q_sb[t2][h2 * 64:(h2 + 1) * 64,
                                                     n * 512:(n + 1) * 512],
                                            start=True, stop=True)
                                    ex = expp.tile([128, 1024], BF16, name="ex",
                                                   tag="ex")
                                    nc.scalar.activation(ex[:], sps[:], AF.Exp,
                                                         scale=0.125)
                                    exs[h2] = ex
                                if prev is not None:
                                    ctx_block(prev[0], prev[1], stop=False)
                                prev = (kt2, exs)
                            ctx_block(prev[0], prev[1], stop=True)

                            # eager psum drain (ctx + den), recip right away
                            for h2 in (0, 1):
                                den_t = expp.tile([128, 512], F32, name="den",
                                                  tag="den", bufs=2)
                                rcp = expp.tile([128, 512], F32R, name="rcp",
                                                tag="rcp", bufs=5)
                                nc.vector.tensor_copy(
                                    ctxa[t2][h2 * 64:(h2 + 1) * 64,
                                             n * 512:(n + 1) * 512],
                                    cps[h2][0:DKH, :])
                                nc.vector.tensor_copy(den_t[0:1, :],
                                                      cps[h2][DKH:VP, :])
                                if n == NQ - 1 and t2 == 1:
                                    # final block: 1/den on the now-idle
                                    # scalar engine (exp(-ln(den)), same
                                    # table) so the DVE queue is free for
                                    # the LN1/transpose tail
                                    dln = expp.tile([128, 512], F32,
                                                    name="den", tag="den",
                                                    bufs=2)
                                    nc.scalar.activation(dln[0:1, :],
                                                         den_t[0:1, :], AF.Ln)
                                    with nc.allow_low_precision(
                                            reason="softmax 1/denom, f32r"):
                                        nc.scalar.activation(
                                            rcp[0:1, :], dln[0:1, :], AF.Exp,
                                            scale=-1.0)
                                else:
                                    with nc.allow_low_precision(
                                            reason="softmax 1/denom, f32r"):
                                        nc.vector.reciprocal(rcp[0:1, :],
                                                             den_t[0:1, :])
                                norm_q.append((t2, h2, n, rcp))
                            lag = 0 if n == NQ - 1 and t2 == 1 else 2
                            while len(norm_q) > lag:
                                emit_norm(norm_q.pop(0))

                            if t2 == 0:
                                if pending_wo is not None:
                                    while norm_q and norm_q[0][2] == pending_wo:
                                        emit_norm(norm_q.pop(0))
                                    emit_wo_rs(pending_wo)
                                    pending_ln.append(pending_wo)
                                    pending_wo = None
                                if len(pending_ln) > 1:
                                    emit_ln1(pending_ln.pop(0))
                        pending_wo = n

                    while norm_q:
                        emit_norm(norm_q.pop(0))
                    emit_wo_rs(pending_wo)
                    pending_ln.append(pending_wo)
                    while len(pending_ln) > 1:
                        emit_ln1(pending_ln.pop(0))
                    att_ps.close()
                    attns_st.close()

                    # ---- tail: overlap last RS with transposes + FFN1-A ----
                    with contextlib.ExitStack() as tail_stack:
                        x1tp = tail_stack.enter_context(
                            tc.tile_pool(name="x1tp", bufs=1))
                        hp = tail_stack.enter_context(
                            tc.tile_pool(name="hp", bufs=1))
                        wstr4 = tail_stack.enter_context(
                            tc.tile_pool(name="wstr4", bufs=8))
                        wstr5 = tail_stack.enter_context(
                            tc.tile_pool(name="wstr5", bufs=8))
                        bp = tail_stack.enter_context(
                            tc.tile_pool(name="bp", bufs=1))
                        ln2c = tail_stack.enter_context(
                            tc.tile_pool(name="ln2c", bufs=1))
                        x1t = [x1tp.tile([128, 512], BF16, name=f"x1t{c}",
                                         tag=f"x1t{c}")
                               for c in range(NDC)]
                        identity = x1tp.tile([128, 128], F32, name="identity")
                        make_identity(nc, identity[:])
                        b1_sb = bp.tile([128, NFT], F32, name="b1_sb")
                        nc.sync.dma_start(out=b1_sb[:], in_=b1m[:, :])
                        gb2_sb = ln2c.tile([128, D], BF16, name="gb2_sb")
                        nc.sync.dma_start(out=gb2_sb[:], in_=gb2[:, :])
                        bb2_sb = ln2c.tile([128, D], F32, name="bb2_sb")
                        nc.sync.dma_start(out=bb2_sb[:], in_=bb2[:, :])
                        b2b_sb = ln2c.tile([128, D], F32, name="b2b_sb")
                        nc.sync.dma_start(out=b2b_sb[:], in_=b2b[:, :])
                        ht = [hp.tile([128, 512], BF16, name=f"ht{t}",
                                      tag=f"ht{t}")
                              for t in range(NFT)]

                        with contextlib.ExitStack() as f1_stack:
                            psT = f1_stack.enter_context(
                                tc.tile_pool(name="psT", bufs=3, space="PSUM"))
                            psF = f1_stack.enter_context(
                                tc.tile_pool(name="psF", bufs=3, space="PSUM"))
                            def transp(i):
                                for c in range(NDC):
                                    ps = psT.tile([128, 128], F32, name="pst",
                                                  tag="pst")
                                    nc.tensor.transpose(
                                        ps[:], x1[i][:, c * 128:(c + 1) * 128],
                                        identity[:])
                                    nc.vector.tensor_copy(
                                        x1t[c][:, i * 128:(i + 1) * 128], ps[:])

                            def ffn1_pass(lo, hi, tag):
                                for t in range(NFT):
                                    wcb = wstr4.tile([128, D], BF16, name="wcb1",
                                                     tag=tag)
                                    nc.sync.dma_start(out=wcb[:], in_=w1b[t])
                                    ps = psF.tile([128, 512], F32, name="psh",
                                                  tag="psh")
                                    for c in range(NDC):
                                        nc.tensor.matmul(
                                            ps[:, lo:hi],
                                            wcb[:, c * 128:(c + 1) * 128],
                                            x1t[c][:, lo:hi],
                                            start=(c == 0), stop=(c == NDC - 1))
                                    nc.scalar.activation(
                                        ht[t][:, lo:hi], ps[:, lo:hi], AF.Relu,
                                        bias=b1_sb[:, t:t + 1])

                            for c in range(NDC):
                                for i in range(3):
                                    ps = psT.tile([128, 128], F32, name="pst",
                                                  tag="pst")
                                    nc.tensor.transpose(
                                        ps[:], x1[i][:, c * 128:(c + 1) * 128],
                                        identity[:])
                                    nc.vector.tensor_copy(
                                        x1t[c][:, i * 128:(i + 1) * 128], ps[:])
                            emit_ln1(pending_ln.pop(0))  # LN1(3): waits RS_3
                            ffn1_pass(0, 384, "wcb1a")   # covers RS_3 + LN1(3)
                            transp(3)
                            ffn1_pass(384, 512, "wcb1b")

                        # ---- FFN2 + LN2 ----
                        with contextlib.ExitStack() as f2_stack:
                            ln2p = f2_stack.enter_context(
                                tc.tile_pool(name="ln2p", bufs=1))
                            psO = f2_stack.enter_context(
                                tc.tile_pool(name="psO", bufs=1, space="PSUM"))
                            fo = [ln2p.tile([128, D], F32, name=f"fo{i}",
                                            tag=f"fo{i}")
                                  for i in range(NQ)]
                            for n2 in range(2):
                                pss = [psO.tile([128, 512], F32,
                                                name=f"pso{i}",
                                                tag=f"pso{i}{n2}")
                                       for i in range(NQ)]
                                for t in range(NFT):
                                    w2c = wstr5.tile([128, 512], BF16,
                                                     name="w2c", tag="w2c")
                                    nc.sync.dma_start(
                                        out=w2c[:],
                                        in_=w2[t * 128:(t + 1) * 128,
                                               n2 * 512:(n2 + 1) * 512])
                                    for i in range(NQ):
                                        nc.tensor.matmul(
                                            pss[i][:],
                                            ht[t][:, i * 128:(i + 1) * 128],
                                            w2c[:],
                                            start=(t == 0), stop=(t == NFT - 1))
                                for i in range(NQ):
                                    nc.vector.tensor_add(
                                        fo[i][:, n2 * 512:(n2 + 1) * 512],
                                        pss[i][:],
                                        b2b_sb[:, n2 * 512:(n2 + 1) * 512])
                            for i in range(NQ):
                                stat = ln2p.tile([128, 4], F32, name="st2",
                                                 tag="st2")
                                cent = ln2p.tile([128, D], F32, name="ce2",
                                                 tag="ce2")
                                sq = ln2p.tile([128, D], BF16, name="sq2",
                                               tag="sq2")
                                nc.vector.tensor_reduce(stat[:, 0:1], fo[i][:],
                                                        mybir.AxisListType.X,
                                                        OP.add)
                                nc.vector.tensor_scalar_mul(stat[:, 1:2],
                                                            stat[:, 0:1],
                                                            1.0 / D)
                                nc.vector.tensor_scalar_sub(cent[:], fo[i][:],
                                                            stat[:, 1:2])
                                nc.vector.scalar_tensor_tensor(
                                    sq[:], fo[i][:], stat[:, 1:2], cent[:],
                                    op0=OP.subtract, op1=OP.mult,
                                    accum_out=stat[:, 2:3])
                                nc.scalar.activation(stat[:, 3:4], stat[:, 2:3],
                                                     AF.Ln, bias=epsb[:, 0:1],
                                                     scale=1.0 / D)
                                nc.scalar.activation(stat[:, 0:1], stat[:, 3:4],
                                                     AF.Exp, scale=-0.5)
                                t1 = ln2p.tile([128, D], F32, name="t1b",
                                               tag="t1b")
                                nc.vector.scalar_tensor_tensor(
                                    t1[:], cent[:], stat[:, 0:1], gb2_sb[:],
                                    op0=OP.mult, op1=OP.mult)
                                t2t = ln2p.tile([128, D], F32, name="t2b",
                                                tag="t2b")
                                nc.vector.tensor_add(t2t[:], x1[i][:],
                                                     bb2_sb[:])
                                xo = ln2p.tile([128, D], F32, name="xo",
                                               tag="xo")
                                nc.vector.tensor_add(xo[:], t1[:], t2t[:])
                                nc.sync.dma_start(
                                    out=out[i * 128:(i + 1) * 128, :],
                                    in_=xo[:])

    nc.finalize()
    return nc


_NC = None


def _get_nc():
    global _NC
    if _NC is None:
        _NC = _build()
    return _NC


def _host_prep(batch_X, padding_mask, W_Q, W_K, W_V, W_O, W1, b1, W2, b2,
               gamma1, beta1, gamma2, beta2):
    import ml_dtypes
    f = np.float32
    bf = ml_dtypes.bfloat16
    X = np.asarray(batch_X, f)
    pm = np.asarray(padding_mask)

    def colblocks(w, nt, dt=f):
        nd = w.shape[0] // 128
        return np.ascontiguousarray(
            np.asarray(w, f).reshape(nd, 128, nt, 128).transpose(2, 1, 0, 3)
            .astype(dt)).reshape(nt, 128, w.shape[0])

    shared = {
        "w1b": colblocks(np.asarray(W1, f), NFT, bf),
        "w2": np.ascontiguousarray(np.asarray(W2, f).astype(bf)),
        "b1m": np.ascontiguousarray(np.asarray(b1, f).reshape(NFT, 128).T),
        "gb1": np.ascontiguousarray(
            np.broadcast_to(np.asarray(gamma1, f), (128, D)).astype(bf)),
        "gb2": np.ascontiguousarray(
            np.broadcast_to(np.asarray(gamma2, f), (128, D)).astype(bf)),
        "bb2": np.ascontiguousarray(
            np.broadcast_to(np.asarray(beta2, f), (128, D))),
        "b2b": np.ascontiguousarray(
            np.broadcast_to(np.asarray(b2, f), (128, D))),
    }
    WQ, WK, WV, WO = (np.asarray(w, f) for w in (W_Q, W_K, W_V, W_O))
    be1 = np.asarray(beta1, f)
    in_maps = []
    for core in range(NCORES):
        b = core // 4
        r = core % 4
        hs = slice(r * 256, (r + 1) * 256)
        rows = np.concatenate(
            [np.arange(j * 512 + r * 128, j * 512 + r * 128 + 128)
             for j in range(NQ)])
        m = dict(shared)
        m["xt"] = np.ascontiguousarray(X[b].T.astype(bf))
        m["wqh"] = colblocks(WQ[:, hs], 2, bf)
        m["wkh"] = colblocks(WK[:, hs], 2, bf)
        m["wvh"] = np.ascontiguousarray(WV[:, hs].astype(bf))
        m["woh"] = np.ascontiguousarray(
            WO[hs, :].reshape(2, 128, D).astype(bf))
        m["keep"] = np.ascontiguousarray(
            (pm[b][rows] != 0).astype(f).reshape(NQ, 128).T)
        m["xqb"] = np.ascontiguousarray(X[b][rows] + be1)
        in_maps.append(m)
    return in_maps


def kernel(**inputs):
    nc = _get_nc()
    in_maps = _host_prep(**inputs)
    res = bass_utils.run_bass_kernel_spmd(nc, in_maps, list(range(NCORES)))
    out = np.empty((B, S, D), np.float32)
    for core in range(NCORES):
        b = core // 4
        r = core % 4
        for j in range(NQ):
            out[b, j * 512 + r * 128:j * 512 + r * 128 + 128] = \
                res.results[core]["out"][j * 128:(j + 1) * 128]
    return out



# revision 17
# speedup vs baseline: 1.1039x; 1.1039x over previous
"""Trainium2 Bass kernel for a dense transformer encoder layer — v3.

Sharding: tensor-parallel attention within each batch group of 4 cores.
Core (b=core//4, r=core%4) computes Q/K/V and attention for heads
4r..4r+3 over ALL 2048 rows of batch b, multiplies by its 256-row slice
of W_O (per-core weight content), and a pipelined 4-core ReduceScatter
(bf16) per q-quarter sums the partial attention outputs, delivering
each core its own row-tiles rank-independently.  LN1/residual, FFN and
LN2 then run row-parallel.

v3 changes vs v2.1:
- scores PSUM rotation deepened to 3 tiles (6 banks): breaks the
  exp -> bank-reuse -> scores dependency cycle that held the PE at a
  ~2us/step period and kept the HAM clock gate at K=4/8 (1.2 GHz).
- W_O partials and the 1/den broadcast matmuls borrow slots from the
  scores rotation (tag "sps") instead of a dedicated psW pool, freeing
  the 2 banks the deeper scores rotation needs.
- W_O emitted as 2 accumulating matmuls of N=1024 per row-tile
  (moving operand bf16 allows 1024 free dim) instead of 4 of N=512:
  halves the serialized LDWEIGHTS count.
- Q/K projections reordered (weights stationary per (t2,qk,c), two
  N=1024 matmuls per load over the 4 quarters) - 64 instead of 128
  matmuls.
- FFN2: w2 fully resident in SBUF (DMAed on the gpsimd queue during
  attention), i-outer loop with one [128,1024] psum per row-tile and
  32 accumulating N=1024 matmuls; LN2(i) and the output DMA overlap
  the next row-tile's matmul stream.  FFN2-A (row-tiles 0-2) runs
  before FFN1 pass B so only FFN2-B depends on the last-quarter ht.
"""

import contextlib

import numpy as np

import concourse.bass as bass
import concourse.mybir as mybir
from concourse import bacc
from concourse import bass_utils
from concourse.masks import make_identity
from concourse.tile import TileContext

F32 = mybir.dt.float32
F32R = mybir.dt.float32r
BF16 = mybir.dt.bfloat16
AF = mybir.ActivationFunctionType
OP = mybir.AluOpType

B, S, D, H, DKH, DFF = 2, 2048, 1024, 16, 64, 4096
EPS = 1e-6
NCORES = 8
GROUPS = [[0, 1, 2, 3], [4, 5, 6, 7]]
NDC = D // 128          # 8 contraction chunks
NKT = S // 128          # 16 key tiles
NQ = S // 512           # 4 q-quarters
NFT = DFF // 128        # 32 FFN tiles
VP = DKH + 1            # 65
LN_EXP_TABLE = 6        # natural_log_exp_and_others in act_info.json


def _build():
    nc = bacc.Bacc(None, num_devices=NCORES)

    xt = nc.declare_dram_parameter("xt", [D, S], BF16, isOutput=False)
    wqh = nc.declare_dram_parameter("wqh", [2, 128, D], BF16, isOutput=False)
    wkh = nc.declare_dram_parameter("wkh", [2, 128, D], BF16, isOutput=False)
    wvh = nc.declare_dram_parameter("wvh", [D, 256], BF16, isOutput=False)
    woh = nc.declare_dram_parameter("woh", [2, 128, D], BF16, isOutput=False)
    w1b = nc.declare_dram_parameter("w1b", [NFT, 128, D], BF16, isOutput=False)
    w2 = nc.declare_dram_parameter("w2", [DFF, D], BF16, isOutput=False)
    b1m = nc.declare_dram_parameter("b1m", [128, NFT], F32, isOutput=False)
    gb1 = nc.declare_dram_parameter("gb1", [128, D], BF16, isOutput=False)
    gb2 = nc.declare_dram_parameter("gb2", [128, D], BF16, isOutput=False)
    bb2 = nc.declare_dram_parameter("bb2", [128, D], F32, isOutput=False)
    b2b = nc.declare_dram_parameter("b2b", [128, D], F32, isOutput=False)
    keep = nc.declare_dram_parameter("keep", [128, NQ], F32, isOutput=False)
    xqb = nc.declare_dram_parameter("xqb", [512, D], F32, isOutput=False)
    out = nc.declare_dram_parameter("out", [512, D], F32, isOutput=True)

    with TileContext(nc) as tc:
        with tc.tile_pool(name="constp", bufs=1) as constp, \
             tc.tile_pool(name="dramp", bufs=1, space="DRAM") as dramp, \
             tc.tile_pool(name="persist", bufs=1) as persist:
            # pin the act table once: exp/ln/relu/copy all live in it
            nc.scalar.add_instruction(mybir.InstLoadActFuncSet(
                name=f"I-{nc.next_id()}", ins=[], outs=[],
                act_func_set_id=LN_EXP_TABLE))
            epsb = constp.tile([128, 1], F32, name="epsb")
            nc.vector.memset(epsb[:], EPS)
            ones_f = constp.tile([128, 128], F32, name="ones_f")
            nc.vector.memset(ones_f[:], 1.0)
            ones64 = constp.tile([128, 128], F32R, name="ones64")
            nc.vector.tensor_copy(ones64[:], ones_f[:])

            # warmup collective
            wz = constp.tile([128, 8], F32, name="wz")
            nc.vector.memset(wz[:], 0.0)
            biw = dramp.tile([4, 128, 8], F32, name="biw")
            bow = dramp.tile([128, 8], F32, name="bow")
            for g in range(4):
                nc.sync.dma_start(out=biw[g], in_=wz[:])
            nc.gpsimd.collective_compute(
                "ReduceScatter", OP.add, replica_groups=GROUPS,
                ins=[biw[:].opt()], outs=[bow[:].opt()])

            bi = [dramp.tile([4, 128, D], BF16, name=f"bi{j}") for j in range(NQ)]
            bo = [dramp.tile([128, D], BF16, name=f"bo{j}") for j in range(NQ)]

            x1 = [persist.tile([128, D], F32, name=f"x1_{i}", tag=f"x1_{i}")
                  for i in range(NQ)]

            with tc.tile_pool(name="lncp", bufs=1) as lncp, \
                 tc.tile_pool(name="lnp", bufs=1) as lnp:
                attns_st = contextlib.ExitStack()
                attns = attns_st.enter_context(tc.tile_pool(name="attns", bufs=1))
                expp = attns_st.enter_context(tc.tile_pool(name="expp", bufs=4))
                wostg = attns_st.enter_context(tc.tile_pool(name="wostg", bufs=3))
                q_sb = [attns.tile([128, S], BF16, name=f"q{t}", tag=f"q{t}")
                        for t in range(2)]
                k_sb = [attns.tile([128, S], BF16, name=f"k{t}", tag=f"k{t}")
                        for t in range(2)]
                v_sb = [attns.tile([128, 4 * VP], BF16, name=f"v{t}", tag=f"v{t}")
                        for t in range(NKT)]
                ctxa = [attns.tile([128, S], BF16, name=f"ca{t}", tag=f"ca{t}")
                        for t in range(2)]
                wo_sb = [attns.tile([128, D], BF16, name=f"wo{t}", tag=f"wo{t}")
                         for t in range(2)]

                # ---- projections ----
                with tc.tile_pool(name="xthp", bufs=1) as xthp, \
                     tc.tile_pool(name="wproj", bufs=1) as wproj, \
                     tc.tile_pool(name="psP", bufs=3, space="PSUM") as psP, \
                     tc.tile_pool(name="psV", bufs=2, space="PSUM") as psV:
                    wq_sb = [wproj.tile([128, D], BF16, name=f"wq{t}", tag=f"wq{t}")
                             for t in range(2)]
                    wk_sb = [wproj.tile([128, D], BF16, name=f"wk{t}", tag=f"wk{t}")
                             for t in range(2)]
                    wvc = [wproj.tile([128, 256], BF16, name=f"wv{c}", tag=f"wv{c}")
                           for c in range(NDC)]
                    xth = [xthp.tile([128, S], BF16, name=f"xth{c}", tag=f"xth{c}")
                           for c in range(NDC)]
                    # xt chunks first so c=0 matmuls can start ASAP
                    for c in range(NDC):
                        nc.sync.dma_start(out=xth[c][:],
                                          in_=xt[c * 128:(c + 1) * 128, :])
                    for t2 in range(2):
                        nc.sync.dma_start(out=wq_sb[t2][:], in_=wqh[t2])
                        nc.sync.dma_start(out=wk_sb[t2][:], in_=wkh[t2])
                    for c in range(NDC):
                        nc.sync.dma_start(out=wvc[c][:],
                                          in_=wvh[c * 128:(c + 1) * 128, :])
                    for t2 in range(2):
                        nc.sync.dma_start(out=wo_sb[t2][:], in_=woh[t2])

                    for t2 in range(2):
                        for n in range(NQ):
                            for w_sb, dst in ((wq_sb, q_sb), (wk_sb, k_sb)):
                                ps = psP.tile([128, 512], F32, name="psp",
                                              tag="psp")
                                for c in range(NDC):
                                    nc.tensor.matmul(
                                        ps[:], w_sb[t2][:, c * 128:(c + 1) * 128],
                                        xth[c][:, n * 512:(n + 1) * 512],
                                        start=(c == 0), stop=(c == NDC - 1))
                                nc.vector.tensor_copy(
                                    dst[t2][:, n * 512:(n + 1) * 512], ps[:])

                    for kt in range(NKT):
                        vr = v_sb[kt][:].rearrange("p (h c) -> p h c", c=VP)
                        nc.vector.tensor_copy(vr[:, :, DKH], ones_f[:, 0:4])
                    for kt in range(NKT):
                        ps = psV.tile([128, 256], F32, name="psv", tag="psv")
                        for c in range(NDC):
                            nc.tensor.matmul(
                                ps[:], xth[c][:, kt * 128:(kt + 1) * 128],
                                wvc[c][:],
                                start=(c == 0), stop=(c == NDC - 1))
                        vr = v_sb[kt][:].rearrange("p (h c) -> p h c", c=VP)
                        nc.vector.tensor_copy(
                            vr[:, :, 0:DKH],
                            ps[:].rearrange("p (h c) -> p h c", c=DKH))

                # ---- attention + lazy W_O + pipelined ReduceScatter ----
                if True:
                    att_ps = contextlib.ExitStack()
                    psS = att_ps.enter_context(
                        tc.tile_pool(name="psS", bufs=3, space="PSUM"))
                    psC = att_ps.enter_context(
                        tc.tile_pool(name="psC", bufs=2, space="PSUM"))
                    keep_sb = lncp.tile([128, NQ], F32, name="keep_sb")
                    nc.sync.dma_start(out=keep_sb[:], in_=keep[:, :])
                    gb1_sb = lncp.tile([128, D], BF16, name="gb1_sb")
                    nc.sync.dma_start(out=gb1_sb[:], in_=gb1[:, :])
                    xqb_sb = [lncp.tile([128, D], F32, name=f"xqb{i}", tag=f"xqb{i}")
                              for i in range(NQ)]
                    for i in range(NQ):
                        nc.sync.dma_start(out=xqb_sb[i][:],
                                          in_=xqb[i * 128:(i + 1) * 128, :])

                    def emit_norm(p):
                        t2_, h2_, n_, rcp = p
                        rb = psS.tile([128, 1024], F32, name="psn", tag="sps")
                        nc.tensor.matmul(rb[:, 0:512], ones64[0:1, :],
                                         rcp[0:1, :], start=True, stop=True)
                        rbs = expp.tile([128, 512], BF16, name="rbs", tag="rbs",
                                        bufs=2)
                        nc.vector.tensor_copy(
                            rbs[h2_ * 64:(h2_ + 1) * 64, :],
                            rb[h2_ * 64:(h2_ + 1) * 64, 0:512])
                        sl = ctxa[t2_][h2_ * 64:(h2_ + 1) * 64,
                                       n_ * 512:(n_ + 1) * 512]
                        nc.vector.tensor_mul(sl, sl,
                                             rbs[h2_ * 64:(h2_ + 1) * 64, :])

                    def emit_wo_rs(n):
                        for p in range(4):
                            qi = 4 * n + p
                            ps = psS.tile([128, 1024], F32, name="psw",
                                          tag="sps")
                            for n2 in range(2):
                                for t2_ in range(2):
                                    nc.tensor.matmul(
                                        ps[:, n2 * 512:(n2 + 1) * 512],
                                        ctxa[t2_][:, qi * 128:(qi + 1) * 128],
                                        wo_sb[t2_][:, n2 * 512:(n2 + 1) * 512],
                                        start=(t2_ == 0), stop=(t2_ == 1))
                            stg = wostg.tile([128, D], BF16, name="stg",
                                             tag="stg")
                            nc.vector.tensor_copy(stg[:], ps[:])
                            nc.sync.dma_start(out=bi[n][p], in_=stg[:])
                        nc.gpsimd.collective_compute(
                            "ReduceScatter", OP.add, replica_groups=GROUPS,
                            ins=[bi[n][:].opt()], outs=[bo[n][:].opt()])

                    def emit_ln1(j):
                        ao = lnp.tile([128, D], BF16, name="ao", tag="ao")
                        nc.sync.dma_start(out=ao[:], in_=bo[j][:])
                        aom = lnp.tile([128, D], BF16, name="aom", tag="aom")
                        nc.vector.tensor_scalar_mul(aom[:], ao[:],
                                                    keep_sb[:, j:j + 1])
                        stat = lnp.tile([128, 4], F32, name="stat", tag="stat")
                        cent = lnp.tile([128, D], BF16, name="cent", tag="cent")
                        sq = lnp.tile([128, D], BF16, name="sq", tag="sq")
                        nc.vector.tensor_reduce(stat[:, 0:1], aom[:],
                                                mybir.AxisListType.X, OP.add)
                        nc.vector.tensor_scalar_mul(stat[:, 1:2], stat[:, 0:1],
                                                    1.0 / D)
                        nc.vector.tensor_scalar_sub(cent[:], aom[:],
                                                    stat[:, 1:2])
                        nc.vector.scalar_tensor_tensor(
                            sq[:], aom[:], stat[:, 1:2], cent[:],
                            op0=OP.subtract, op1=OP.mult, accum_out=stat[:, 2:3])
                        nc.scalar.activation(stat[:, 3:4], stat[:, 2:3], AF.Ln,
                                             bias=epsb[:, 0:1], scale=1.0 / D)
                        nc.scalar.activation(stat[:, 0:1], stat[:, 3:4], AF.Exp,
                                             scale=-0.5)
                        t1 = lnp.tile([128, D], F32, name="t1", tag="t1")
                        nc.vector.scalar_tensor_tensor(
                            t1[:], cent[:], stat[:, 0:1], gb1_sb[:],
                            op0=OP.mult, op1=OP.mult)
                        nc.vector.tensor_add(x1[j][:], t1[:], xqb_sb[j][:])

                    norm_q = []
                    pending_wo = None   # quarter whose W_O is not yet emitted
                    pending_ln = []     # quarters whose LN1 is not yet emitted
                    for n in range(NQ):
                        for t2 in range(2):
                            cps = {h2: psC.tile([VP, 512], F32, name="cps",
                                                tag="cps") for h2 in (0, 1)}
                            prev = None

                            def ctx_block(pk, pexs, stop):
                                for h2 in (0, 1):
                                    h = 2 * t2 + h2
                                    for u in (0, 1):
                                        kt = 2 * pk + u
                                        nc.tensor.matmul(
                                            cps[h2][:],
                                            v_sb[kt][:, h * VP:(h + 1) * VP],
                                            pexs[h2][:, u * 512:(u + 1) * 512],
                                            start=(pk == 0 and u == 0),
                                            stop=(stop and u == 1))

                            for kt2 in range(NKT // 2):
                                exs = {}
                                for h2 in (0, 1):
                                    sps = psS.tile([128, 1024], F32, name="sps",
                                                   tag="sps")
                                    for u in (0, 1):
                                        kt = 2 * kt2 + u
                                        nc.tensor.matmul(
                                            sps[:, u * 512:(u + 1) * 512],
                                            k_sb[t2][h2 * 64:(h2 + 1) * 64,
                                                     kt * 128:(kt + 1) * 128],
                                            q_sb[t2][h2 * 64:(h2 + 1) * 64,
                                                     n * 512:(n + 1) * 512],
                                            start=True, stop=True)
                                    ex = expp.tile([128, 1024], BF16, name="ex",
                                                   tag="ex")
                                    nc.scalar.activation(ex[:], sps[:], AF.Exp,
                                                         scale=0.125)
                                    exs[h2] = ex
                                if prev is not None:
                                    ctx_block(prev[0], prev[1], stop=False)
                                prev = (kt2, exs)
                            ctx_block(prev[0], prev[1], stop=True)

                            # eager psum drain (ctx + den), recip right away
                            for h2 in (0, 1):
                                den_t = expp.tile([128, 512], F32, name="den",
                                                  tag="den", bufs=2)
                                rcp = expp.tile([128, 512], F32R, name="rcp",
                                                tag="rcp", bufs=5)
                                nc.vector.tensor_copy(
                                    ctxa[t2][h2 * 64:(h2 + 1) * 64,
                                             n * 512:(n + 1) * 512],
                                    cps[h2][0:DKH, :])
                                nc.vector.tensor_copy(den_t[0:1, :],
                                                      cps[h2][DKH:VP, :])
                                if n == NQ - 1 and t2 == 1:
                                    # final block: 1/den on the now-idle
                                    # scalar engine (exp(-ln(den)), same
                                    # table) so the DVE queue is free for
                                    # the LN1/transpose tail
                                    dln = expp.tile([128, 512], F32,
                                                    name="den", tag="den",
                                                    bufs=2)
                                    nc.scalar.activation(dln[0:1, :],
                                                         den_t[0:1, :], AF.Ln)
                                    with nc.allow_low_precision(
                                            reason="softmax 1/denom, f32r"):
                                        nc.scalar.activation(
                                            rcp[0:1, :], dln[0:1, :], AF.Exp,
                                            scale=-1.0)
                                else:
                                    with nc.allow_low_precision(
                                            reason="softmax 1/denom, f32r"):
                                        nc.vector.reciprocal(rcp[0:1, :],
                                                             den_t[0:1, :])
                                norm_q.append((t2, h2, n, rcp))
                            lag = 0 if n == NQ - 1 and t2 == 1 else 2
                            while len(norm_q) > lag:
                                emit_norm(norm_q.pop(0))

                            if t2 == 0:
                                if pending_wo is not None:
                                    while norm_q and norm_q[0][2] == pending_wo:
                                        emit_norm(norm_q.pop(0))
                                    emit_wo_rs(pending_wo)
                                    pending_ln.append(pending_wo)
                                    pending_wo = None
                                if len(pending_ln) > 1:
                                    emit_ln1(pending_ln.pop(0))
                        pending_wo = n

                    while norm_q:
                        emit_norm(norm_q.pop(0))
                    emit_wo_rs(pending_wo)
                    pending_ln.append(pending_wo)
                    while len(pending_ln) > 1:
                        emit_ln1(pending_ln.pop(0))
                    att_ps.close()
                    attns_st.close()

                    # ---- tail: overlap last RS with transposes + FFN1-A ----
                    with contextlib.ExitStack() as tail_stack:
                        # w2 resident for FFN2: DMAed up-front on the gpsimd
                        # queue; the 8MB lands during FFN1 pass A.
                        w2p = tail_stack.enter_context(
                            tc.tile_pool(name="w2p", bufs=1))
                        w2row = [w2p.tile([128, D], BF16, name=f"w2r{t}",
                                          tag=f"w2r{t}") for t in range(NFT)]
                        for t in range(NFT):
                            nc.gpsimd.dma_start(out=w2row[t][:],
                                                in_=w2[t * 128:(t + 1) * 128, :])
                        x1tp = tail_stack.enter_context(
                            tc.tile_pool(name="x1tp", bufs=1))
                        hp = tail_stack.enter_context(
                            tc.tile_pool(name="hp", bufs=1))
                        wstr4 = tail_stack.enter_context(
                            tc.tile_pool(name="wstr4", bufs=4))
                        bp = tail_stack.enter_context(
                            tc.tile_pool(name="bp", bufs=1))
                        ln2c = tail_stack.enter_context(
                            tc.tile_pool(name="ln2c", bufs=1))
                        psF = tail_stack.enter_context(
                            tc.tile_pool(name="psF", bufs=3, space="PSUM"))
                        x1t = [x1tp.tile([128, 512], BF16, name=f"x1t{c}",
                                         tag=f"x1t{c}")
                               for c in range(NDC)]
                        identity = x1tp.tile([128, 128], F32, name="identity")
                        make_identity(nc, identity[:])
                        b1_sb = bp.tile([128, NFT], F32, name="b1_sb")
                        nc.sync.dma_start(out=b1_sb[:], in_=b1m[:, :])
                        gb2_sb = ln2c.tile([128, D], BF16, name="gb2_sb")
                        nc.sync.dma_start(out=gb2_sb[:], in_=gb2[:, :])
                        bb2_sb = ln2c.tile([128, D], F32, name="bb2_sb")
                        nc.sync.dma_start(out=bb2_sb[:], in_=bb2[:, :])
                        b2b_sb = ln2c.tile([128, D], F32, name="b2b_sb")
                        nc.sync.dma_start(out=b2b_sb[:], in_=b2b[:, :])
                        ht = [hp.tile([128, 512], BF16, name=f"ht{t}",
                                      tag=f"ht{t}")
                              for t in range(NFT)]

                        def ffn1_pass(lo, hi, tag):
                            for t in range(NFT):
                                wcb = wstr4.tile([128, D], BF16, name="wcb1",
                                                 tag=tag)
                                nc.sync.dma_start(out=wcb[:], in_=w1b[t])
                                ps = psF.tile([128, 512], F32, name="psh",
                                              tag="psh")
                                for c in range(NDC):
                                    nc.tensor.matmul(
                                        ps[:, lo:hi],
                                        wcb[:, c * 128:(c + 1) * 128],
                                        x1t[c][:, lo:hi],
                                        start=(c == 0), stop=(c == NDC - 1))
                                nc.scalar.activation(
                                    ht[t][:, lo:hi], ps[:, lo:hi], AF.Relu,
                                    bias=b1_sb[:, t:t + 1])

                        with contextlib.ExitStack() as t_stack:
                            psT = t_stack.enter_context(
                                tc.tile_pool(name="psT", bufs=3, space="PSUM"))

                            def transp(i):
                                for c in range(NDC):
                                    ps = psT.tile([128, 128], F32, name="pst",
                                                  tag="pst")
                                    nc.tensor.transpose(
                                        ps[:], x1[i][:, c * 128:(c + 1) * 128],
                                        identity[:])
                                    nc.vector.tensor_copy(
                                        x1t[c][:, i * 128:(i + 1) * 128], ps[:])

                            for c in range(NDC):
                                for i in range(3):
                                    ps = psT.tile([128, 128], F32, name="pst",
                                                  tag="pst")
                                    nc.tensor.transpose(
                                        ps[:], x1[i][:, c * 128:(c + 1) * 128],
                                        identity[:])
                                    nc.vector.tensor_copy(
                                        x1t[c][:, i * 128:(i + 1) * 128], ps[:])
                            emit_ln1(pending_ln.pop(0))  # LN1(3): waits RS_3
                            ffn1_pass(0, 384, "wcb1a")   # covers RS_3 + LN1(3)
                            transp(3)

                        # ---- FFN2 (i-outer, resident w2) + FFN1-B + LN2 ----
                        with contextlib.ExitStack() as f2_stack:
                            ln2p = f2_stack.enter_context(
                                tc.tile_pool(name="ln2p", bufs=1))
                            psO = f2_stack.enter_context(
                                tc.tile_pool(name="psO", bufs=2, space="PSUM"))

                            def emit_ln2(i, fo):
                                stat = ln2p.tile([128, 4], F32, name="st2",
                                                 tag="st2", bufs=2)
                                cent = ln2p.tile([128, D], F32, name="ce2",
                                                 tag="ce2")
                                sq = ln2p.tile([128, D], BF16, name="sq2",
                                               tag="sq2")
                                nc.vector.tensor_reduce(stat[:, 0:1], fo[:],
                                                        mybir.AxisListType.X,
                                                        OP.add)
                                nc.vector.tensor_scalar_mul(stat[:, 1:2],
                                                            stat[:, 0:1],
                                                            1.0 / D)
                                nc.vector.tensor_scalar_sub(cent[:], fo[:],
                                                            stat[:, 1:2])
                                nc.vector.scalar_tensor_tensor(
                                    sq[:], fo[:], stat[:, 1:2], cent[:],
                                    op0=OP.subtract, op1=OP.mult,
                                    accum_out=stat[:, 2:3])
                                nc.scalar.activation(stat[:, 3:4], stat[:, 2:3],
                                                     AF.Ln, bias=epsb[:, 0:1],
                                                     scale=1.0 / D)
                                nc.scalar.activation(stat[:, 0:1], stat[:, 3:4],
                                                     AF.Exp, scale=-0.5)
                                t1 = ln2p.tile([128, D], F32, name="t1b",
                                               tag="t1b")
                                nc.vector.scalar_tensor_tensor(
                                    t1[:], cent[:], stat[:, 0:1], gb2_sb[:],
                                    op0=OP.mult, op1=OP.mult)
                                xo = ln2p.tile([128, D], F32, name="xo",
                                               tag="xo", bufs=2)
                                nc.vector.tensor_add(xo[:], t1[:], x1[i][:])
                                nc.vector.tensor_add(xo[:], xo[:], bb2_sb[:])
                                nc.sync.dma_start(
                                    out=out[i * 128:(i + 1) * 128, :],
                                    in_=xo[:])

                            def ffn2_row(i):
                                ps = psO.tile([128, D], F32, name="pso",
                                              tag="pso")
                                for t in range(NFT):
                                    for n2 in range(2):
                                        nc.tensor.matmul(
                                            ps[:, n2 * 512:(n2 + 1) * 512],
                                            ht[t][:, i * 128:(i + 1) * 128],
                                            w2row[t][:, n2 * 512:(n2 + 1) * 512],
                                            start=(t == 0), stop=(t == NFT - 1))
                                fo = ln2p.tile([128, D], F32, name="fo",
                                               tag="fo", bufs=2)
                                nc.vector.tensor_add(fo[:], ps[:], b2b_sb[:])
                                emit_ln2(i, fo)

                            for i in range(3):
                                ffn2_row(i)
                            ffn1_pass(384, 512, "wcb1b")
                            ffn2_row(3)

    nc.finalize()
    return nc


_NC = None


def _get_nc():
    global _NC
    if _NC is None:
        _NC = _build()
    return _NC


def _host_prep(batch_X, padding_mask, W_Q, W_K, W_V, W_O, W1, b1, W2, b2,
               gamma1, beta1, gamma2, beta2):
    import ml_dtypes
    f = np.float32
    bf = ml_dtypes.bfloat16
    X = np.asarray(batch_X, f)
    pm = np.asarray(padding_mask)

    def colblocks(w, nt, dt=f):
        nd = w.shape[0] // 128
        return np.ascontiguousarray(
            np.asarray(w, f).reshape(nd, 128, nt, 128).transpose(2, 1, 0, 3)
            .astype(dt)).reshape(nt, 128, w.shape[0])

    shared = {
        "w1b": colblocks(np.asarray(W1, f), NFT, bf),
        "w2": np.ascontiguousarray(np.asarray(W2, f).astype(bf)),
        "b1m": np.ascontiguousarray(np.asarray(b1, f).reshape(NFT, 128).T),
        "gb1": np.ascontiguousarray(
            np.broadcast_to(np.asarray(gamma1, f), (128, D)).astype(bf)),
        "gb2": np.ascontiguousarray(
            np.broadcast_to(np.asarray(gamma2, f), (128, D)).astype(bf)),
        "bb2": np.ascontiguousarray(
            np.broadcast_to(np.asarray(beta2, f), (128, D))),
        "b2b": np.ascontiguousarray(
            np.broadcast_to(np.asarray(b2, f), (128, D))),
    }
    WQ, WK, WV, WO = (np.asarray(w, f) for w in (W_Q, W_K, W_V, W_O))
    be1 = np.asarray(beta1, f)
    in_maps = []
    for core in range(NCORES):
        b = core // 4
        r = core % 4
        hs = slice(r * 256, (r + 1) * 256)
        rows = np.concatenate(
            [np.arange(j * 512 + r * 128, j * 512 + r * 128 + 128)
             for j in range(NQ)])
        m = dict(shared)
        m["xt"] = np.ascontiguousarray(X[b].T.astype(bf))
        m["wqh"] = colblocks(WQ[:, hs], 2, bf)
        m["wkh"] = colblocks(WK[:, hs], 2, bf)
        m["wvh"] = np.ascontiguousarray(WV[:, hs].astype(bf))
        m["woh"] = np.ascontiguousarray(
            WO[hs, :].reshape(2, 128, D).astype(bf))
        m["keep"] = np.ascontiguousarray(
            (pm[b][rows] != 0).astype(f).reshape(NQ, 128).T)
        m["xqb"] = np.ascontiguousarray(X[b][rows] + be1)
        in_maps.append(m)
    return in_maps


def kernel(**inputs):
    nc = _get_nc()
    in_maps = _host_prep(**inputs)
    res = bass_utils.run_bass_kernel_spmd(nc, in_maps, list(range(NCORES)))
    out = np.empty((B, S, D), np.float32)
    for core in range(NCORES):
        b = core // 4
        r = core % 4
        for j in range(NQ):
            out[b, j * 512 + r * 128:j * 512 + r * 128 + 128] = \
                res.results[core]["out"][j * 128:(j + 1) * 128]
    return out


# revision 24
# speedup vs baseline: 1.1508x; 1.0425x over previous
"""Trainium2 Bass kernel for a dense transformer encoder layer — v3.

Sharding: tensor-parallel attention within each batch group of 4 cores.
Core (b=core//4, r=core%4) computes Q/K/V and attention for heads
4r..4r+3 over ALL 2048 rows of batch b, multiplies by its 256-row slice
of W_O (per-core weight content), and a pipelined 4-core ReduceScatter
(bf16) per q-quarter sums the partial attention outputs, delivering
each core its own row-tiles rank-independently.  LN1/residual, FFN and
LN2 then run row-parallel.

v3 changes vs v2.1:
- scores PSUM rotation deepened to 3 tiles (6 banks): breaks the
  exp -> bank-reuse -> scores dependency cycle that held the PE at a
  ~2us/step period and kept the HAM clock gate at K=4/8 (1.2 GHz).
- W_O partials and the 1/den broadcast matmuls borrow slots from the
  scores rotation (tag "sps") instead of a dedicated psW pool, freeing
  the 2 banks the deeper scores rotation needs.
- W_O emitted as 2 accumulating matmuls of N=1024 per row-tile
  (moving operand bf16 allows 1024 free dim) instead of 4 of N=512:
  halves the serialized LDWEIGHTS count.
- Q/K projections reordered (weights stationary per (t2,qk,c), two
  N=1024 matmuls per load over the 4 quarters) - 64 instead of 128
  matmuls.
- FFN2: w2 fully resident in SBUF (DMAed on the gpsimd queue during
  attention), i-outer loop with one [128,1024] psum per row-tile and
  32 accumulating N=1024 matmuls; LN2(i) and the output DMA overlap
  the next row-tile's matmul stream.  FFN2-A (row-tiles 0-2) runs
  before FFN1 pass B so only FFN2-B depends on the last-quarter ht.
"""

import contextlib

import numpy as np

import concourse.bass as bass
import concourse.mybir as mybir
from concourse import bacc
from concourse import bass_utils
from concourse.masks import make_identity
from concourse.tile import TileContext

F32 = mybir.dt.float32
F32R = mybir.dt.float32r
BF16 = mybir.dt.bfloat16
AF = mybir.ActivationFunctionType
OP = mybir.AluOpType

B, S, D, H, DKH, DFF = 2, 2048, 1024, 16, 64, 4096
EPS = 1e-6
NCORES = 8
GROUPS = [[0, 1, 2, 3], [4, 5, 6, 7]]
NDC = D // 128          # 8 contraction chunks
NKT = S // 128          # 16 key tiles
NQ = S // 512           # 4 q-quarters
NFT = DFF // 128        # 32 FFN tiles
VP = DKH + 1            # 65
LN_EXP_TABLE = 6        # natural_log_exp_and_others in act_info.json


def _build():
    nc = bacc.Bacc(None, num_devices=NCORES)

    xt = nc.declare_dram_parameter("xt", [D, S], BF16, isOutput=False)
    wqh = nc.declare_dram_parameter("wqh", [2, 128, D], BF16, isOutput=False)
    wkh = nc.declare_dram_parameter("wkh", [2, 128, D], BF16, isOutput=False)
    wvh = nc.declare_dram_parameter("wvh", [D, 256], BF16, isOutput=False)
    woh = nc.declare_dram_parameter("woh", [2, 128, D], BF16, isOutput=False)
    w1b = nc.declare_dram_parameter("w1b", [NFT, 128, D], BF16, isOutput=False)
    w2 = nc.declare_dram_parameter("w2", [DFF, D], BF16, isOutput=False)
    b1m = nc.declare_dram_parameter("b1m", [128, NFT], F32, isOutput=False)
    gb1 = nc.declare_dram_parameter("gb1", [128, D], BF16, isOutput=False)
    gb2 = nc.declare_dram_parameter("gb2", [128, D], BF16, isOutput=False)
    bb2 = nc.declare_dram_parameter("bb2", [128, D], F32, isOutput=False)
    b2b = nc.declare_dram_parameter("b2b", [128, D], F32, isOutput=False)
    keep = nc.declare_dram_parameter("keep", [128, NQ], F32, isOutput=False)
    xqb = nc.declare_dram_parameter("xqb", [512, D], F32, isOutput=False)
    out = nc.declare_dram_parameter("out", [512, D], F32, isOutput=True)

    with TileContext(nc) as tc:
        with tc.tile_pool(name="constp", bufs=1) as constp, \
             tc.tile_pool(name="dramp", bufs=1, space="DRAM") as dramp, \
             tc.tile_pool(name="persist", bufs=1) as persist:
            # pin the act table once: exp/ln/relu/copy all live in it
            nc.scalar.add_instruction(mybir.InstLoadActFuncSet(
                name=f"I-{nc.next_id()}", ins=[], outs=[],
                act_func_set_id=LN_EXP_TABLE))
            epsb = constp.tile([128, 1], F32, name="epsb")
            nc.vector.memset(epsb[:], EPS)
            ones_f = constp.tile([128, 128], F32, name="ones_f")
            nc.vector.memset(ones_f[:], 1.0)
            ones64 = constp.tile([128, 128], F32R, name="ones64")
            nc.vector.tensor_copy(ones64[:], ones_f[:])
            identity = constp.tile([128, 128], F32, name="identity")
            make_identity(nc, identity[:])

            # warmup collective
            wz = constp.tile([128, 8], F32, name="wz")
            nc.vector.memset(wz[:], 0.0)
            biw = dramp.tile([4, 128, 8], F32, name="biw")
            bow = dramp.tile([128, 8], F32, name="bow")
            for g in range(4):
                nc.sync.dma_start(out=biw[g], in_=wz[:])
            nc.gpsimd.collective_compute(
                "ReduceScatter", OP.add, replica_groups=GROUPS,
                ins=[biw[:].opt()], outs=[bow[:].opt()])

            bi = [dramp.tile([4, 128, D], BF16, name=f"bi{j}") for j in range(NQ)]
            bo = [dramp.tile([128, D], BF16, name=f"bo{j}") for j in range(NQ)]

            x1 = [persist.tile([128, D], F32, name=f"x1_{i}", tag=f"x1_{i}")
                  for i in range(NQ)]

            with tc.tile_pool(name="lncp", bufs=1) as lncp, \
                 tc.tile_pool(name="lnp", bufs=1) as lnp:
                attns_st = contextlib.ExitStack()
                attns = attns_st.enter_context(tc.tile_pool(name="attns", bufs=1))
                expp = attns_st.enter_context(tc.tile_pool(name="expp", bufs=4))
                wostg = attns_st.enter_context(tc.tile_pool(name="wostg", bufs=3))
                q_sb = [attns.tile([128, S], BF16, name=f"q{t}", tag=f"q{t}")
                        for t in range(2)]
                k_sb = [attns.tile([128, S], BF16, name=f"k{t}", tag=f"k{t}")
                        for t in range(2)]
                v_sb = [attns.tile([128, 4 * VP], BF16, name=f"v{t}", tag=f"v{t}")
                        for t in range(NKT)]
                ctxa = [attns.tile([128, S], BF16, name=f"ca{t}", tag=f"ca{t}")
                        for t in range(2)]
                wo_sb = [attns.tile([128, D], BF16, name=f"wo{t}", tag=f"wo{t}")
                         for t in range(2)]

                # ---- projections ----
                with tc.tile_pool(name="xthp", bufs=1) as xthp, \
                     tc.tile_pool(name="wproj", bufs=1) as wproj, \
                     tc.tile_pool(name="psP", bufs=8, space="PSUM") as psP:
                    wq_sb = [wproj.tile([128, D], BF16, name=f"wq{t}", tag=f"wq{t}")
                             for t in range(2)]
                    wk_sb = [wproj.tile([128, D], BF16, name=f"wk{t}", tag=f"wk{t}")
                             for t in range(2)]
                    wvc = [wproj.tile([128, 256], BF16, name=f"wv{c}", tag=f"wv{c}")
                           for c in range(NDC)]
                    xth = [xthp.tile([128, S], BF16, name=f"xth{c}", tag=f"xth{c}")
                           for c in range(NDC)]
                    # t2=0 weights first, then xt chunks split across two DMA
                    # queues, so the c-outer matmul groups below chase the
                    # arriving chunks instead of waiting for all of xt.
                    nc.sync.dma_start(out=wq_sb[0][:], in_=wqh[0])
                    nc.scalar.dma_start(out=wk_sb[0][:], in_=wkh[0])
                    for c in range(NDC):
                        eng = nc.sync if c % 2 == 0 else nc.scalar
                        eng.dma_start(out=xth[c][:],
                                      in_=xt[c * 128:(c + 1) * 128, :])
                    nc.sync.dma_start(out=wq_sb[1][:], in_=wqh[1])
                    nc.scalar.dma_start(out=wk_sb[1][:], in_=wkh[1])
                    for c in range(NDC):
                        eng = nc.sync if c % 2 == 0 else nc.scalar
                        eng.dma_start(out=wvc[c][:],
                                      in_=wvh[c * 128:(c + 1) * 128, :])
                    for t2 in range(2):
                        nc.sync.dma_start(out=wo_sb[t2][:], in_=woh[t2])

                    # Q/K: c-outer over an 8-psum group per t2 so the first
                    # matmuls start as soon as chunk 0 lands.
                    for t2 in range(2):
                        pss = {}
                        for n in range(NQ):
                            for qk in range(2):
                                pss[(n, qk)] = psP.tile([128, 512], F32,
                                                        name="psp", tag="psp")
                        for c in range(NDC):
                            for n in range(NQ):
                                for qk, w_sb in ((0, wq_sb), (1, wk_sb)):
                                    nc.tensor.matmul(
                                        pss[(n, qk)][:],
                                        w_sb[t2][:, c * 128:(c + 1) * 128],
                                        xth[c][:, n * 512:(n + 1) * 512],
                                        start=(c == 0), stop=(c == NDC - 1))
                        for n in range(NQ):
                            for qk, dst in ((0, q_sb), (1, k_sb)):
                                nc.vector.tensor_copy(
                                    dst[t2][:, n * 512:(n + 1) * 512],
                                    pss[(n, qk)][:])

                    for kt in range(NKT):
                        vr = v_sb[kt][:].rearrange("p (h c) -> p h c", c=VP)
                        nc.vector.tensor_copy(vr[:, :, DKH], ones_f[:, 0:4])
                    for kt in range(NKT):
                        ps = psP.tile([128, 512], F32, name="psv", tag="psp")
                        for c in range(NDC):
                            nc.tensor.matmul(
                                ps[:, 0:256], xth[c][:, kt * 128:(kt + 1) * 128],
                                wvc[c][:],
                                start=(c == 0), stop=(c == NDC - 1))
                        vr = v_sb[kt][:].rearrange("p (h c) -> p h c", c=VP)
                        nc.vector.tensor_copy(
                            vr[:, :, 0:DKH],
                            ps[:, 0:256].rearrange("p (h c) -> p h c", c=DKH))

                # ---- attention + lazy W_O + pipelined ReduceScatter ----
                if True:
                    att_ps = contextlib.ExitStack()
                    psS = att_ps.enter_context(
                        tc.tile_pool(name="psS", bufs=3, space="PSUM"))
                    psC = att_ps.enter_context(
                        tc.tile_pool(name="psC", bufs=2, space="PSUM"))
                    keep_sb = lncp.tile([128, NQ], F32, name="keep_sb")
                    nc.sync.dma_start(out=keep_sb[:], in_=keep[:, :])
                    gb1_sb = lncp.tile([128, D], BF16, name="gb1_sb")
                    nc.sync.dma_start(out=gb1_sb[:], in_=gb1[:, :])
                    xqb_sb = [lncp.tile([128, D], F32, name=f"xqb{i}", tag=f"xqb{i}")
                              for i in range(NQ)]
                    for i in range(NQ):
                        nc.sync.dma_start(out=xqb_sb[i][:],
                                          in_=xqb[i * 128:(i + 1) * 128, :])

                    def emit_norm(p):
                        t2_, h2_, n_, rcp = p
                        rb = psS.tile([128, 1024], F32, name="psn", tag="sps")
                        nc.tensor.matmul(rb[:, 0:512], ones64[0:1, :],
                                         rcp[0:1, :], start=True, stop=True)
                        rbs = expp.tile([128, 512], BF16, name="rbs", tag="rbs",
                                        bufs=2)
                        nc.vector.tensor_copy(
                            rbs[h2_ * 64:(h2_ + 1) * 64, :],
                            rb[h2_ * 64:(h2_ + 1) * 64, 0:512])
                        sl = ctxa[t2_][h2_ * 64:(h2_ + 1) * 64,
                                       n_ * 512:(n_ + 1) * 512]
                        nc.vector.tensor_mul(sl, sl,
                                             rbs[h2_ * 64:(h2_ + 1) * 64, :])

                    def emit_wo_rs(n):
                        for p in range(4):
                            qi = 4 * n + p
                            ps = psS.tile([128, 1024], F32, name="psw",
                                          tag="sps")
                            for n2 in range(2):
                                for t2_ in range(2):
                                    nc.tensor.matmul(
                                        ps[:, n2 * 512:(n2 + 1) * 512],
                                        ctxa[t2_][:, qi * 128:(qi + 1) * 128],
                                        wo_sb[t2_][:, n2 * 512:(n2 + 1) * 512],
                                        start=(t2_ == 0), stop=(t2_ == 1))
                            stg = wostg.tile([128, D], BF16, name="stg",
                                             tag="stg")
                            nc.vector.tensor_copy(stg[:], ps[:])
                            nc.sync.dma_start(out=bi[n][p], in_=stg[:])
                        nc.gpsimd.collective_compute(
                            "ReduceScatter", OP.add, replica_groups=GROUPS,
                            ins=[bi[n][:].opt()], outs=[bo[n][:].opt()])

                    def emit_ln1(j):
                        ao = lnp.tile([128, D], BF16, name="ao", tag="ao")
                        nc.sync.dma_start(out=ao[:], in_=bo[j][:])
                        aom = lnp.tile([128, D], BF16, name="aom", tag="aom")
                        nc.vector.tensor_scalar_mul(aom[:], ao[:],
                                                    keep_sb[:, j:j + 1])
                        stat = lnp.tile([128, 4], F32, name="stat", tag="stat")
                        cent = lnp.tile([128, D], BF16, name="cent", tag="cent")
                        sq = lnp.tile([128, D], BF16, name="sq", tag="sq")
                        nc.vector.tensor_reduce(stat[:, 0:1], aom[:],
                                                mybir.AxisListType.X, OP.add)
                        nc.vector.tensor_scalar_mul(stat[:, 1:2], stat[:, 0:1],
                                                    1.0 / D)
                        nc.vector.tensor_scalar_sub(cent[:], aom[:],
                                                    stat[:, 1:2])
                        nc.vector.scalar_tensor_tensor(
                            sq[:], aom[:], stat[:, 1:2], cent[:],
                            op0=OP.subtract, op1=OP.mult, accum_out=stat[:, 2:3])
                        nc.scalar.activation(stat[:, 3:4], stat[:, 2:3], AF.Ln,
                                             bias=epsb[:, 0:1], scale=1.0 / D)
                        nc.scalar.activation(stat[:, 0:1], stat[:, 3:4], AF.Exp,
                                             scale=-0.5)
                        t1 = lnp.tile([128, D], F32, name="t1", tag="t1")
                        nc.vector.scalar_tensor_tensor(
                            t1[:], cent[:], stat[:, 0:1], gb1_sb[:],
                            op0=OP.mult, op1=OP.mult)
                        nc.vector.tensor_add(x1[j][:], t1[:], xqb_sb[j][:])

                    norm_q = []
                    pending_wo = None   # quarter whose W_O is not yet emitted
                    pending_ln = []     # quarters whose LN1 is not yet emitted
                    for n in range(NQ):
                        for t2 in range(2):
                            cps = {h2: psC.tile([VP, 512], F32, name="cps",
                                                tag="cps") for h2 in (0, 1)}
                            prev = None

                            def ctx_block(pk, pexs, stop):
                                for h2 in (0, 1):
                                    h = 2 * t2 + h2
                                    for u in (0, 1):
                                        kt = 2 * pk + u
                                        nc.tensor.matmul(
                                            cps[h2][:],
                                            v_sb[kt][:, h * VP:(h + 1) * VP],
                                            pexs[h2][:, u * 512:(u + 1) * 512],
                                            start=(pk == 0 and u == 0),
                                            stop=(stop and u == 1))

                            for kt2 in range(NKT // 2):
                                exs = {}
                                for h2 in (0, 1):
                                    sps = psS.tile([128, 1024], F32, name="sps",
                                                   tag="sps")
                                    for u in (0, 1):
                                        kt = 2 * kt2 + u
                                        nc.tensor.matmul(
                                            sps[:, u * 512:(u + 1) * 512],
                                            k_sb[t2][h2 * 64:(h2 + 1) * 64,
                                                     kt * 128:(kt + 1) * 128],
                                            q_sb[t2][h2 * 64:(h2 + 1) * 64,
                                                     n * 512:(n + 1) * 512],
                                            start=True, stop=True)
                                    ex = expp.tile([128, 1024], BF16, name="ex",
                                                   tag="ex")
                                    nc.scalar.activation(ex[:], sps[:], AF.Exp,
                                                         scale=0.125)
                                    exs[h2] = ex
                                if prev is not None:
                                    ctx_block(prev[0], prev[1], stop=False)
                                prev = (kt2, exs)
                            ctx_block(prev[0], prev[1], stop=True)

                            # eager psum drain (ctx + den), recip right away
                            for h2 in (0, 1):
                                den_t = expp.tile([128, 512], F32, name="den",
                                                  tag="den", bufs=2)
                                rcp = expp.tile([128, 512], F32R, name="rcp",
                                                tag="rcp", bufs=5)
                                nc.vector.tensor_copy(
                                    ctxa[t2][h2 * 64:(h2 + 1) * 64,
                                             n * 512:(n + 1) * 512],
                                    cps[h2][0:DKH, :])
                                nc.vector.tensor_copy(den_t[0:1, :],
                                                      cps[h2][DKH:VP, :])
                                if n == NQ - 1 and t2 == 1:
                                    # final block: 1/den on the now-idle
                                    # scalar engine (exp(-ln(den)), same
                                    # table) so the DVE queue is free for
                                    # the LN1/transpose tail
                                    dln = expp.tile([128, 512], F32,
                                                    name="den", tag="den",
                                                    bufs=2)
                                    nc.scalar.activation(dln[0:1, :],
                                                         den_t[0:1, :], AF.Ln)
                                    with nc.allow_low_precision(
                                            reason="softmax 1/denom, f32r"):
                                        nc.scalar.activation(
                                            rcp[0:1, :], dln[0:1, :], AF.Exp,
                                            scale=-1.0)
                                else:
                                    with nc.allow_low_precision(
                                            reason="softmax 1/denom, f32r"):
                                        nc.vector.reciprocal(rcp[0:1, :],
                                                             den_t[0:1, :])
                                norm_q.append((t2, h2, n, rcp))
                            lag = 0 if n == NQ - 1 and t2 == 1 else 2
                            while len(norm_q) > lag:
                                emit_norm(norm_q.pop(0))

                            if t2 == 0:
                                if pending_wo is not None:
                                    while norm_q and norm_q[0][2] == pending_wo:
                                        emit_norm(norm_q.pop(0))
                                    emit_wo_rs(pending_wo)
                                    pending_ln.append(pending_wo)
                                    pending_wo = None
                                if len(pending_ln) > 1:
                                    emit_ln1(pending_ln.pop(0))
                        pending_wo = n

                    while norm_q:
                        emit_norm(norm_q.pop(0))
                    emit_wo_rs(pending_wo)
                    pending_ln.append(pending_wo)
                    while len(pending_ln) > 1:
                        emit_ln1(pending_ln.pop(0))
                    att_ps.close()
                    attns_st.close()

                    # ---- tail: overlap last RS with transposes + FFN1-A ----
                    with contextlib.ExitStack() as tail_stack:
                        # w2 resident for FFN2: DMAed one row-block per
                        # FFN1-A iteration on the gpsimd queue (a single
                        # up-front burst of 8MB stalls the gpsimd engine for
                        # ~45us and everything queued behind it).
                        w2p = tail_stack.enter_context(
                            tc.tile_pool(name="w2p", bufs=1))
                        w2row = [w2p.tile([128, D], BF16, name=f"w2r{t}",
                                          tag=f"w2r{t}") for t in range(NFT)]
                        x1tp = tail_stack.enter_context(
                            tc.tile_pool(name="x1tp", bufs=1))
                        hp = tail_stack.enter_context(
                            tc.tile_pool(name="hp", bufs=1))
                        wstr4 = tail_stack.enter_context(
                            tc.tile_pool(name="wstr4", bufs=4))
                        bp = tail_stack.enter_context(
                            tc.tile_pool(name="bp", bufs=1))
                        ln2c = tail_stack.enter_context(
                            tc.tile_pool(name="ln2c", bufs=1))
                        psF = tail_stack.enter_context(
                            tc.tile_pool(name="psF", bufs=3, space="PSUM"))
                        x1t = [x1tp.tile([128, 512], BF16, name=f"x1t{c}",
                                         tag=f"x1t{c}")
                               for c in range(NDC)]
                        b1_sb = bp.tile([128, NFT], F32, name="b1_sb")
                        nc.sync.dma_start(out=b1_sb[:], in_=b1m[:, :])
                        gb2_sb = ln2c.tile([128, D], BF16, name="gb2_sb")
                        nc.sync.dma_start(out=gb2_sb[:], in_=gb2[:, :])
                        bb2_sb = ln2c.tile([128, D], F32, name="bb2_sb")
                        nc.sync.dma_start(out=bb2_sb[:], in_=bb2[:, :])
                        b2b_sb = ln2c.tile([128, D], F32, name="b2b_sb")
                        nc.sync.dma_start(out=b2b_sb[:], in_=b2b[:, :])
                        ht = [hp.tile([128, 512], BF16, name=f"ht{t}",
                                      tag=f"ht{t}")
                              for t in range(NFT)]

                        def ffn1_pass(lo, hi, tag, w2dma=False):
                            for t in range(NFT):
                                if w2dma:
                                    nc.gpsimd.dma_start(
                                        out=w2row[t][:],
                                        in_=w2[t * 128:(t + 1) * 128, :])
                                wcb = wstr4.tile([128, D], BF16, name="wcb1",
                                                 tag=tag)
                                nc.sync.dma_start(out=wcb[:], in_=w1b[t])
                                ps = psF.tile([128, 512], F32, name="psh",
                                              tag="psh")
                                for c in range(NDC):
                                    nc.tensor.matmul(
                                        ps[:, lo:hi],
                                        wcb[:, c * 128:(c + 1) * 128],
                                        x1t[c][:, lo:hi],
                                        start=(c == 0), stop=(c == NDC - 1))
                                nc.scalar.activation(
                                    ht[t][:, lo:hi], ps[:, lo:hi], AF.Relu,
                                    bias=b1_sb[:, t:t + 1])

                        with contextlib.ExitStack() as t_stack:
                            psT = t_stack.enter_context(
                                tc.tile_pool(name="psT", bufs=3, space="PSUM"))

                            def transp(i):
                                for c in range(NDC):
                                    ps = psT.tile([128, 128], F32, name="pst",
                                                  tag="pst")
                                    nc.tensor.transpose(
                                        ps[:], x1[i][:, c * 128:(c + 1) * 128],
                                        identity[:])
                                    nc.vector.tensor_copy(
                                        x1t[c][:, i * 128:(i + 1) * 128], ps[:])

                            for c in range(NDC):
                                for i in range(3):
                                    ps = psT.tile([128, 128], F32, name="pst",
                                                  tag="pst")
                                    nc.tensor.transpose(
                                        ps[:], x1[i][:, c * 128:(c + 1) * 128],
                                        identity[:])
                                    nc.vector.tensor_copy(
                                        x1t[c][:, i * 128:(i + 1) * 128], ps[:])
                            emit_ln1(pending_ln.pop(0))  # LN1(3): waits RS_3
                            # covers RS_3 + LN1(3); also streams in w2
                            ffn1_pass(0, 384, "wcb1a", w2dma=True)
                            transp(3)

                        # ---- FFN2 (i-outer, resident w2) + FFN1-B + LN2 ----
                        with contextlib.ExitStack() as f2_stack:
                            ln2p = f2_stack.enter_context(
                                tc.tile_pool(name="ln2p", bufs=1))
                            psO = f2_stack.enter_context(
                                tc.tile_pool(name="psO", bufs=2, space="PSUM"))

                            def emit_ln2(i, fo):
                                stat = ln2p.tile([128, 4], F32, name="st2",
                                                 tag="st2", bufs=2)
                                cent = ln2p.tile([128, D], F32, name="ce2",
                                                 tag="ce2")
                                sq = ln2p.tile([128, D], BF16, name="sq2",
                                               tag="sq2")
                                nc.vector.tensor_reduce(stat[:, 0:1], fo[:],
                                                        mybir.AxisListType.X,
                                                        OP.add)
                                nc.vector.tensor_scalar_mul(stat[:, 1:2],
                                                            stat[:, 0:1],
                                                            1.0 / D)
                                nc.vector.tensor_scalar_sub(cent[:], fo[:],
                                                            stat[:, 1:2])
                                nc.vector.scalar_tensor_tensor(
                                    sq[:], fo[:], stat[:, 1:2], cent[:],
                                    op0=OP.subtract, op1=OP.mult,
                                    accum_out=stat[:, 2:3])
                                nc.scalar.activation(stat[:, 3:4], stat[:, 2:3],
                                                     AF.Ln, bias=epsb[:, 0:1],
                                                     scale=1.0 / D)
                                nc.scalar.activation(stat[:, 0:1], stat[:, 3:4],
                                                     AF.Exp, scale=-0.5)
                                t1 = ln2p.tile([128, D], F32, name="t1b",
                                               tag="t1b")
                                nc.vector.scalar_tensor_tensor(
                                    t1[:], cent[:], stat[:, 0:1], gb2_sb[:],
                                    op0=OP.mult, op1=OP.mult)
                                xo = ln2p.tile([128, D], F32, name="xo",
                                               tag="xo", bufs=2)
                                nc.vector.tensor_add(xo[:], t1[:], x1[i][:])
                                nc.vector.tensor_add(xo[:], xo[:], bb2_sb[:])
                                nc.sync.dma_start(
                                    out=out[i * 128:(i + 1) * 128, :],
                                    in_=xo[:])

                            def ffn2_row(i):
                                ps = psO.tile([128, D], F32, name="pso",
                                              tag="pso")
                                for t in range(NFT):
                                    for n2 in range(2):
                                        nc.tensor.matmul(
                                            ps[:, n2 * 512:(n2 + 1) * 512],
                                            ht[t][:, i * 128:(i + 1) * 128],
                                            w2row[t][:, n2 * 512:(n2 + 1) * 512],
                                            start=(t == 0), stop=(t == NFT - 1))
                                fo = ln2p.tile([128, D], F32, name="fo",
                                               tag="fo", bufs=2)
                                nc.vector.tensor_add(fo[:], ps[:], b2b_sb[:])
                                emit_ln2(i, fo)

                            for i in range(3):
                                ffn2_row(i)
                            ffn1_pass(384, 512, "wcb1b")
                            ffn2_row(3)

    nc.finalize()
    return nc


_NC = None


def _get_nc():
    global _NC
    if _NC is None:
        _NC = _build()
    return _NC


def _host_prep(batch_X, padding_mask, W_Q, W_K, W_V, W_O, W1, b1, W2, b2,
               gamma1, beta1, gamma2, beta2):
    import ml_dtypes
    f = np.float32
    bf = ml_dtypes.bfloat16
    X = np.asarray(batch_X, f)
    pm = np.asarray(padding_mask)

    def colblocks(w, nt, dt=f):
        nd = w.shape[0] // 128
        return np.ascontiguousarray(
            np.asarray(w, f).reshape(nd, 128, nt, 128).transpose(2, 1, 0, 3)
            .astype(dt)).reshape(nt, 128, w.shape[0])

    shared = {
        "w1b": colblocks(np.asarray(W1, f), NFT, bf),
        "w2": np.ascontiguousarray(np.asarray(W2, f).astype(bf)),
        "b1m": np.ascontiguousarray(np.asarray(b1, f).reshape(NFT, 128).T),
        "gb1": np.ascontiguousarray(
            np.broadcast_to(np.asarray(gamma1, f), (128, D)).astype(bf)),
        "gb2": np.ascontiguousarray(
            np.broadcast_to(np.asarray(gamma2, f), (128, D)).astype(bf)),
        "bb2": np.ascontiguousarray(
            np.broadcast_to(np.asarray(beta2, f), (128, D))),
        "b2b": np.ascontiguousarray(
            np.broadcast_to(np.asarray(b2, f), (128, D))),
    }
    WQ, WK, WV, WO = (np.asarray(w, f) for w in (W_Q, W_K, W_V, W_O))
    be1 = np.asarray(beta1, f)
    in_maps = []
    for core in range(NCORES):
        b = core // 4
        r = core % 4
        hs = slice(r * 256, (r + 1) * 256)
        rows = np.concatenate(
            [np.arange(j * 512 + r * 128, j * 512 + r * 128 + 128)
             for j in range(NQ)])
        m = dict(shared)
        m["xt"] = np.ascontiguousarray(X[b].T.astype(bf))
        m["wqh"] = colblocks(WQ[:, hs], 2, bf)
        m["wkh"] = colblocks(WK[:, hs], 2, bf)
        m["wvh"] = np.ascontiguousarray(WV[:, hs].astype(bf))
        m["woh"] = np.ascontiguousarray(
            WO[hs, :].reshape(2, 128, D).astype(bf))
        m["keep"] = np.ascontiguousarray(
            (pm[b][rows] != 0).astype(f).reshape(NQ, 128).T)
        m["xqb"] = np.ascontiguousarray(X[b][rows] + be1)
        in_maps.append(m)
    return in_maps


def kernel(**inputs):
    nc = _get_nc()
    in_maps = _host_prep(**inputs)
    res = bass_utils.run_bass_kernel_spmd(nc, in_maps, list(range(NCORES)))
    out = np.empty((B, S, D), np.float32)
    for core in range(NCORES):
        b = core // 4
        r = core % 4
        for j in range(NQ):
            out[b, j * 512 + r * 128:j * 512 + r * 128 + 128] = \
                res.results[core]["out"][j * 128:(j + 1) * 128]
    return out


# revision 34
# speedup vs baseline: 1.1831x; 1.0281x over previous
"""Trainium2 Bass kernel for a dense transformer encoder layer — v3.

Sharding: tensor-parallel attention within each batch group of 4 cores.
Core (b=core//4, r=core%4) computes Q/K/V and attention for heads
4r..4r+3 over ALL 2048 rows of batch b, multiplies by its 256-row slice
of W_O (per-core weight content), and a pipelined 4-core ReduceScatter
(bf16) per q-quarter sums the partial attention outputs, delivering
each core its own row-tiles rank-independently.  LN1/residual, FFN and
LN2 then run row-parallel.

v3 changes vs v2.1:
- scores PSUM rotation deepened to 3 tiles (6 banks): breaks the
  exp -> bank-reuse -> scores dependency cycle that held the PE at a
  ~2us/step period and kept the HAM clock gate at K=4/8 (1.2 GHz).
- W_O partials and the 1/den broadcast matmuls borrow slots from the
  scores rotation (tag "sps") instead of a dedicated psW pool, freeing
  the 2 banks the deeper scores rotation needs.
- W_O emitted as 2 accumulating matmuls of N=1024 per row-tile
  (moving operand bf16 allows 1024 free dim) instead of 4 of N=512:
  halves the serialized LDWEIGHTS count.
- Q/K projections reordered (weights stationary per (t2,qk,c), two
  N=1024 matmuls per load over the 4 quarters) - 64 instead of 128
  matmuls.
- FFN2: w2 fully resident in SBUF (DMAed on the gpsimd queue during
  attention), i-outer loop with one [128,1024] psum per row-tile and
  32 accumulating N=1024 matmuls; LN2(i) and the output DMA overlap
  the next row-tile's matmul stream.  FFN2-A (row-tiles 0-2) runs
  before FFN1 pass B so only FFN2-B depends on the last-quarter ht.
"""

import contextlib

import numpy as np

import concourse.bass as bass
import concourse.mybir as mybir
from concourse import bacc
from concourse import bass_utils
from concourse.masks import make_identity
from concourse.tile import TileContext

F32 = mybir.dt.float32
F32R = mybir.dt.float32r
BF16 = mybir.dt.bfloat16
AF = mybir.ActivationFunctionType
OP = mybir.AluOpType

B, S, D, H, DKH, DFF = 2, 2048, 1024, 16, 64, 4096
EPS = 1e-6
NCORES = 8
GROUPS = [[0, 1, 2, 3], [4, 5, 6, 7]]
NDC = D // 128          # 8 contraction chunks
NKT = S // 128          # 16 key tiles
NQ = S // 512           # 4 q-quarters
NFT = DFF // 128        # 32 FFN tiles
VP = DKH + 1            # 65
LN_EXP_TABLE = 6        # natural_log_exp_and_others in act_info.json


def _build():
    nc = bacc.Bacc(None, num_devices=NCORES)

    xt = nc.declare_dram_parameter("xt", [D, S], BF16, isOutput=False)
    wqh = nc.declare_dram_parameter("wqh", [2, 128, D], BF16, isOutput=False)
    wkh = nc.declare_dram_parameter("wkh", [2, 128, D], BF16, isOutput=False)
    wvh = nc.declare_dram_parameter("wvh", [D, 256], BF16, isOutput=False)
    woh = nc.declare_dram_parameter("woh", [2, 128, D], BF16, isOutput=False)
    w1b = nc.declare_dram_parameter("w1b", [NFT, 128, D], BF16, isOutput=False)
    w2 = nc.declare_dram_parameter("w2", [DFF, D], BF16, isOutput=False)
    b1m = nc.declare_dram_parameter("b1m", [128, NFT], F32, isOutput=False)
    gb1 = nc.declare_dram_parameter("gb1", [128, D], BF16, isOutput=False)
    gb2 = nc.declare_dram_parameter("gb2", [128, D], BF16, isOutput=False)
    bb2 = nc.declare_dram_parameter("bb2", [128, D], F32, isOutput=False)
    b2b = nc.declare_dram_parameter("b2b", [128, D], F32, isOutput=False)
    keep = nc.declare_dram_parameter("keep", [128, NQ], F32, isOutput=False)
    xqb = nc.declare_dram_parameter("xqb", [512, D], F32, isOutput=False)
    out = nc.declare_dram_parameter("out", [512, D], F32, isOutput=True)

    with TileContext(nc) as tc:
        with tc.tile_pool(name="constp", bufs=1) as constp, \
             tc.tile_pool(name="dramp", bufs=1, space="DRAM") as dramp, \
             tc.tile_pool(name="persist", bufs=1) as persist:
            # pin the act table once: exp/ln/relu/copy all live in it
            nc.scalar.add_instruction(mybir.InstLoadActFuncSet(
                name=f"I-{nc.next_id()}", ins=[], outs=[],
                act_func_set_id=LN_EXP_TABLE))
            epsb = constp.tile([128, 1], F32, name="epsb")
            nc.vector.memset(epsb[:], EPS)
            ones_f = constp.tile([128, 128], F32, name="ones_f")
            nc.vector.memset(ones_f[:], 1.0)
            ones64 = constp.tile([128, 128], F32R, name="ones64")
            nc.vector.tensor_copy(ones64[:], ones_f[:])
            identity = constp.tile([128, 128], F32, name="identity")
            make_identity(nc, identity[:])

            # warmup collective
            wz = constp.tile([128, 8], F32, name="wz")
            nc.vector.memset(wz[:], 0.0)
            biw = dramp.tile([4, 128, 8], F32, name="biw")
            bow = dramp.tile([128, 8], F32, name="bow")
            for g in range(4):
                nc.sync.dma_start(out=biw[g], in_=wz[:])
            nc.gpsimd.collective_compute(
                "ReduceScatter", OP.add, replica_groups=GROUPS,
                ins=[biw[:].opt()], outs=[bow[:].opt()])

            bi = [dramp.tile([4, 128, D], BF16, name=f"bi{j}") for j in range(NQ)]
            bo = [dramp.tile([128, D], BF16, name=f"bo{j}") for j in range(NQ)]

            x1 = [persist.tile([128, D], F32, name=f"x1_{i}", tag=f"x1_{i}")
                  for i in range(NQ)]

            with tc.tile_pool(name="lncp", bufs=1) as lncp, \
                 tc.tile_pool(name="lnp", bufs=1) as lnp:
                attns_st = contextlib.ExitStack()
                attns = attns_st.enter_context(tc.tile_pool(name="attns", bufs=1))
                expp = attns_st.enter_context(tc.tile_pool(name="expp", bufs=4))
                wostg = attns_st.enter_context(tc.tile_pool(name="wostg", bufs=2))
                q_sb = [attns.tile([128, S], BF16, name=f"q{t}", tag=f"q{t}")
                        for t in range(2)]
                k_sb = [attns.tile([128, S], BF16, name=f"k{t}", tag=f"k{t}")
                        for t in range(2)]
                v_sb = [attns.tile([128, 4 * VP], BF16, name=f"v{t}", tag=f"v{t}")
                        for t in range(NKT)]
                ctxa = [attns.tile([128, S], BF16, name=f"ca{t}", tag=f"ca{t}")
                        for t in range(2)]
                wo_sb = [attns.tile([128, D], BF16, name=f"wo{t}", tag=f"wo{t}")
                         for t in range(2)]

                # ---- projections ----
                with tc.tile_pool(name="xthp", bufs=1) as xthp, \
                     tc.tile_pool(name="wproj", bufs=1) as wproj, \
                     tc.tile_pool(name="psP", bufs=8, space="PSUM") as psP:
                    wq_sb = [wproj.tile([128, D], BF16, name=f"wq{t}", tag=f"wq{t}")
                             for t in range(2)]
                    wk_sb = [wproj.tile([128, D], BF16, name=f"wk{t}", tag=f"wk{t}")
                             for t in range(2)]
                    wvc = [wproj.tile([128, 256], BF16, name=f"wv{c}", tag=f"wv{c}")
                           for c in range(NDC)]
                    xth = [xthp.tile([128, S], BF16, name=f"xth{c}", tag=f"xth{c}")
                           for c in range(NDC)]
                    # t2=0 weights first, then xt chunks split across two DMA
                    # queues, so the c-outer matmul groups below chase the
                    # arriving chunks instead of waiting for all of xt.
                    nc.sync.dma_start(out=wq_sb[0][:], in_=wqh[0])
                    nc.scalar.dma_start(out=wk_sb[0][:], in_=wkh[0])
                    for c in range(NDC):
                        eng = nc.sync if c % 2 == 0 else nc.scalar
                        eng.dma_start(out=xth[c][:],
                                      in_=xt[c * 128:(c + 1) * 128, :])
                    nc.sync.dma_start(out=wq_sb[1][:], in_=wqh[1])
                    nc.scalar.dma_start(out=wk_sb[1][:], in_=wkh[1])
                    for c in range(NDC):
                        eng = nc.sync if c % 2 == 0 else nc.scalar
                        eng.dma_start(out=wvc[c][:],
                                      in_=wvh[c * 128:(c + 1) * 128, :])
                    for t2 in range(2):
                        nc.sync.dma_start(out=wo_sb[t2][:], in_=woh[t2])

                    # Q/K: c-outer over an 8-psum group per t2 so the first
                    # matmuls start as soon as chunk 0 lands.
                    for t2 in range(2):
                        pss = {}
                        for n in range(NQ):
                            for qk in range(2):
                                pss[(n, qk)] = psP.tile([128, 512], F32,
                                                        name="psp", tag="psp")
                        for c in range(NDC):
                            for n in range(NQ):
                                for qk, w_sb in ((0, wq_sb), (1, wk_sb)):
                                    nc.tensor.matmul(
                                        pss[(n, qk)][:],
                                        w_sb[t2][:, c * 128:(c + 1) * 128],
                                        xth[c][:, n * 512:(n + 1) * 512],
                                        start=(c == 0), stop=(c == NDC - 1))
                        for n in range(NQ):
                            for qk, dst in ((0, q_sb), (1, k_sb)):
                                nc.vector.tensor_copy(
                                    dst[t2][:, n * 512:(n + 1) * 512],
                                    pss[(n, qk)][:])

                    for kt in range(NKT):
                        vr = v_sb[kt][:].rearrange("p (h c) -> p h c", c=VP)
                        nc.vector.tensor_copy(vr[:, :, DKH], ones_f[:, 0:4])
                    for kt in range(NKT):
                        ps = psP.tile([128, 512], F32, name="psv", tag="psp")
                        for c in range(NDC):
                            nc.tensor.matmul(
                                ps[:, 0:256], xth[c][:, kt * 128:(kt + 1) * 128],
                                wvc[c][:],
                                start=(c == 0), stop=(c == NDC - 1))
                        vr = v_sb[kt][:].rearrange("p (h c) -> p h c", c=VP)
                        nc.vector.tensor_copy(
                            vr[:, :, 0:DKH],
                            ps[:, 0:256].rearrange("p (h c) -> p h c", c=DKH))

                # ---- attention + lazy W_O + pipelined ReduceScatter ----
                if True:
                    att_ps = contextlib.ExitStack()
                    psS = att_ps.enter_context(
                        tc.tile_pool(name="psS", bufs=3, space="PSUM"))
                    psC = att_ps.enter_context(
                        tc.tile_pool(name="psC", bufs=2, space="PSUM"))
                    keep_sb = lncp.tile([128, NQ], F32, name="keep_sb")
                    nc.sync.dma_start(out=keep_sb[:], in_=keep[:, :])
                    gb1_sb = lncp.tile([128, D], BF16, name="gb1_sb")
                    nc.sync.dma_start(out=gb1_sb[:], in_=gb1[:, :])
                    xqb_t = {}

                    def emit_norm(p):
                        t2_, h2_, n_, rcp = p
                        rb = psS.tile([128, 1024], F32, name="psn", tag="sps")
                        nc.tensor.matmul(rb[:, 0:512], ones64[0:1, :],
                                         rcp[0:1, :], start=True, stop=True)
                        rbs = expp.tile([128, 512], BF16, name="rbs", tag="rbs",
                                        bufs=2)
                        nc.vector.tensor_copy(
                            rbs[h2_ * 64:(h2_ + 1) * 64, :],
                            rb[h2_ * 64:(h2_ + 1) * 64, 0:512])
                        sl = ctxa[t2_][h2_ * 64:(h2_ + 1) * 64,
                                       n_ * 512:(n_ + 1) * 512]
                        nc.vector.tensor_mul(sl, sl,
                                             rbs[h2_ * 64:(h2_ + 1) * 64, :])

                    def emit_wo_p(n, p):
                        qi = 4 * n + p
                        ps = psS.tile([128, 1024], F32, name="psw",
                                      tag="sps")
                        for n2 in range(2):
                            for t2_ in range(2):
                                nc.tensor.matmul(
                                    ps[:, n2 * 512:(n2 + 1) * 512],
                                    ctxa[t2_][:, qi * 128:(qi + 1) * 128],
                                    wo_sb[t2_][:, n2 * 512:(n2 + 1) * 512],
                                    start=(t2_ == 0), stop=(t2_ == 1))
                        stg = wostg.tile([128, D], BF16, name="stg",
                                         tag="stg")
                        nc.vector.tensor_copy(stg[:], ps[:])
                        nc.sync.dma_start(out=bi[n][p], in_=stg[:])

                    def emit_rs(n):
                        nc.gpsimd.collective_compute(
                            "ReduceScatter", OP.add, replica_groups=GROUPS,
                            ins=[bi[n][:].opt()], outs=[bo[n][:].opt()])
                        xq = lncp.tile([128, D], F32, name="xqb", tag="xqb",
                                       bufs=2)
                        nc.sync.dma_start(out=xq[:],
                                          in_=xqb[n * 128:(n + 1) * 128, :])
                        xqb_t[n] = xq

                    def emit_ln1(j):
                        ao = lnp.tile([128, D], BF16, name="ao", tag="ao")
                        nc.sync.dma_start(out=ao[:], in_=bo[j][:])
                        nc.vector.tensor_scalar_mul(ao[:], ao[:],
                                                    keep_sb[:, j:j + 1])
                        stat = lnp.tile([128, 4], F32, name="stat", tag="stat")
                        cent = lnp.tile([128, D], BF16, name="cent", tag="cent")
                        sq = lnp.tile([128, D], BF16, name="sq", tag="sq")
                        nc.vector.tensor_reduce(stat[:, 0:1], ao[:],
                                                mybir.AxisListType.X, OP.add)
                        nc.vector.tensor_scalar_mul(stat[:, 1:2], stat[:, 0:1],
                                                    1.0 / D)
                        nc.vector.tensor_scalar_sub(cent[:], ao[:],
                                                    stat[:, 1:2])
                        nc.vector.scalar_tensor_tensor(
                            sq[:], ao[:], stat[:, 1:2], cent[:],
                            op0=OP.subtract, op1=OP.mult, accum_out=stat[:, 2:3])
                        nc.scalar.activation(stat[:, 3:4], stat[:, 2:3], AF.Ln,
                                             bias=epsb[:, 0:1], scale=1.0 / D)
                        nc.scalar.activation(stat[:, 0:1], stat[:, 3:4], AF.Exp,
                                             scale=-0.5)
                        t1 = lnp.tile([128, D], F32, name="t1", tag="t1")
                        nc.vector.scalar_tensor_tensor(
                            t1[:], cent[:], stat[:, 0:1], gb1_sb[:],
                            op0=OP.mult, op1=OP.mult)
                        nc.vector.tensor_add(x1[j][:], t1[:],
                                             xqb_t.pop(j)[:])

                    norm_q = []
                    pending_wo = None   # quarter whose W_O is not yet emitted
                    pending_ln = []     # quarters whose LN1 is not yet emitted
                    for n in range(NQ):
                        for t2 in range(2):
                            cps = {h2: psC.tile([VP, 512], F32, name="cps",
                                                tag="cps") for h2 in (0, 1)}
                            prev = None

                            def ctx_block(pk, pexs, stop):
                                for h2 in (0, 1):
                                    h = 2 * t2 + h2
                                    for u in (0, 1):
                                        kt = 2 * pk + u
                                        nc.tensor.matmul(
                                            cps[h2][:],
                                            v_sb[kt][:, h * VP:(h + 1) * VP],
                                            pexs[h2][:, u * 512:(u + 1) * 512],
                                            start=(pk == 0 and u == 0),
                                            stop=(stop and u == 1))

                            for kt2 in range(NKT // 2):
                                exs = {}
                                for h2 in (0, 1):
                                    sps = psS.tile([128, 1024], F32, name="sps",
                                                   tag="sps")
                                    for u in (0, 1):
                                        kt = 2 * kt2 + u
                                        nc.tensor.matmul(
                                            sps[:, u * 512:(u + 1) * 512],
                                            k_sb[t2][h2 * 64:(h2 + 1) * 64,
                                                     kt * 128:(kt + 1) * 128],
                                            q_sb[t2][h2 * 64:(h2 + 1) * 64,
                                                     n * 512:(n + 1) * 512],
                                            start=True, stop=True)
                                    ex = expp.tile([128, 1024], BF16, name="ex",
                                                   tag="ex")
                                    nc.scalar.activation(ex[:], sps[:], AF.Exp,
                                                         scale=0.125)
                                    exs[h2] = ex
                                if prev is not None:
                                    ctx_block(prev[0], prev[1], stop=False)
                                prev = (kt2, exs)
                                # boundary work, spread through the block so
                                # the scores/exp pipeline never sees a burst:
                                # the previous quarter's norms, W_O tiles and
                                # RS trigger ride in one-per-iteration.
                                if kt2 == 2:
                                    while norm_q:
                                        emit_norm(norm_q.pop(0))
                                if t2 == 0 and pending_wo is not None:
                                    if 3 <= kt2 <= 6:
                                        emit_wo_p(pending_wo, kt2 - 3)
                                    if kt2 == 6:
                                        emit_rs(pending_wo)
                                        pending_ln.append(pending_wo)
                                        pending_wo = None
                                if t2 == 1 and kt2 == 5 and len(pending_ln) > 1:
                                    emit_ln1(pending_ln.pop(0))
                            ctx_block(prev[0], prev[1], stop=True)

                            # eager psum drain (ctx + den); 1/den on DVE for
                            # the t2=0 block (lag hides it), on the scalar
                            # engine (exp(-ln(den)), same table) for the t2=1
                            # block where ACT idles across the quarter
                            # boundary while DVE drains.
                            for h2 in (0, 1):
                                nc.vector.tensor_copy(
                                    ctxa[t2][h2 * 64:(h2 + 1) * 64,
                                             n * 512:(n + 1) * 512],
                                    cps[h2][0:DKH, :])
                            for h2 in (0, 1):
                                den_t = expp.tile([128, 512], F32, name="den",
                                                  tag="den", bufs=2)
                                rcp = expp.tile([128, 512], F32R, name="rcp",
                                                tag="rcp", bufs=3)
                                nc.vector.tensor_copy(den_t[0:1, :],
                                                      cps[h2][DKH:VP, :])
                                if t2 == 1:
                                    dln = expp.tile([128, 512], F32,
                                                    name="den", tag="den",
                                                    bufs=2)
                                    nc.scalar.activation(dln[0:1, :],
                                                         den_t[0:1, :], AF.Ln)
                                    with nc.allow_low_precision(
                                            reason="softmax 1/denom, f32r"):
                                        nc.scalar.activation(
                                            rcp[0:1, :], dln[0:1, :], AF.Exp,
                                            scale=-1.0)
                                else:
                                    with nc.allow_low_precision(
                                            reason="softmax 1/denom, f32r"):
                                        nc.vector.reciprocal(rcp[0:1, :],
                                                             den_t[0:1, :])
                                norm_q.append((t2, h2, n, rcp))
                        pending_wo = n

                    while norm_q:
                        emit_norm(norm_q.pop(0))
                    for p in range(4):
                        emit_wo_p(pending_wo, p)
                    emit_rs(pending_wo)
                    pending_ln.append(pending_wo)
                    while len(pending_ln) > 1:
                        emit_ln1(pending_ln.pop(0))
                    att_ps.close()
                    attns_st.close()

                    # ---- tail: overlap last RS with transposes + FFN1-A ----
                    with contextlib.ExitStack() as tail_stack:
                        # w2 resident for FFN2: DMAed one row-block per
                        # FFN1-A iteration on the gpsimd queue (a single
                        # up-front burst of 8MB stalls the gpsimd engine for
                        # ~45us and everything queued behind it).
                        w2p = tail_stack.enter_context(
                            tc.tile_pool(name="w2p", bufs=1))
                        w2row = [w2p.tile([128, D], BF16, name=f"w2r{t}",
                                          tag=f"w2r{t}") for t in range(NFT)]
                        x1tp = tail_stack.enter_context(
                            tc.tile_pool(name="x1tp", bufs=1))
                        hp = tail_stack.enter_context(
                            tc.tile_pool(name="hp", bufs=1))
                        wstr4 = tail_stack.enter_context(
                            tc.tile_pool(name="wstr4", bufs=4))
                        bp = tail_stack.enter_context(
                            tc.tile_pool(name="bp", bufs=1))
                        ln2c = tail_stack.enter_context(
                            tc.tile_pool(name="ln2c", bufs=1))
                        psF = tail_stack.enter_context(
                            tc.tile_pool(name="psF", bufs=3, space="PSUM"))
                        x1t = [x1tp.tile([128, 512], BF16, name=f"x1t{c}",
                                         tag=f"x1t{c}")
                               for c in range(NDC)]
                        b1_sb = bp.tile([128, NFT], F32, name="b1_sb")
                        nc.sync.dma_start(out=b1_sb[:], in_=b1m[:, :])
                        gb2_sb = ln2c.tile([128, D], BF16, name="gb2_sb")
                        nc.sync.dma_start(out=gb2_sb[:], in_=gb2[:, :])
                        bb2_sb = ln2c.tile([128, D], F32, name="bb2_sb")
                        nc.sync.dma_start(out=bb2_sb[:], in_=bb2[:, :])
                        b2b_sb = ln2c.tile([128, D], F32, name="b2b_sb")
                        nc.sync.dma_start(out=b2b_sb[:], in_=b2b[:, :])
                        ht = [hp.tile([128, 512], BF16, name=f"ht{t}",
                                      tag=f"ht{t}")
                              for t in range(NFT)]

                        def ffn1_pass(lo, hi, tag, w2dma=False):
                            for t in range(NFT):
                                if w2dma:
                                    nc.gpsimd.dma_start(
                                        out=w2row[t][:],
                                        in_=w2[t * 128:(t + 1) * 128, :])
                                wcb = wstr4.tile([128, D], BF16, name="wcb1",
                                                 tag=tag)
                                nc.sync.dma_start(out=wcb[:], in_=w1b[t])
                                ps = psF.tile([128, 512], F32, name="psh",
                                              tag="psh")
                                for c in range(NDC):
                                    nc.tensor.matmul(
                                        ps[:, lo:hi],
                                        wcb[:, c * 128:(c + 1) * 128],
                                        x1t[c][:, lo:hi],
                                        start=(c == 0), stop=(c == NDC - 1))
                                nc.scalar.activation(
                                    ht[t][:, lo:hi], ps[:, lo:hi], AF.Relu,
                                    bias=b1_sb[:, t:t + 1])

                        with contextlib.ExitStack() as t_stack:
                            psT = t_stack.enter_context(
                                tc.tile_pool(name="psT", bufs=3, space="PSUM"))

                            def transp(i):
                                for c in range(NDC):
                                    ps = psT.tile([128, 128], F32, name="pst",
                                                  tag="pst")
                                    nc.tensor.transpose(
                                        ps[:], x1[i][:, c * 128:(c + 1) * 128],
                                        identity[:])
                                    nc.vector.tensor_copy(
                                        x1t[c][:, i * 128:(i + 1) * 128], ps[:])

                            for c in range(NDC):
                                for i in range(3):
                                    ps = psT.tile([128, 128], F32, name="pst",
                                                  tag="pst")
                                    nc.tensor.transpose(
                                        ps[:], x1[i][:, c * 128:(c + 1) * 128],
                                        identity[:])
                                    nc.vector.tensor_copy(
                                        x1t[c][:, i * 128:(i + 1) * 128], ps[:])
                            emit_ln1(pending_ln.pop(0))  # LN1(3): waits RS_3
                            # covers RS_3 + LN1(3); also streams in w2
                            ffn1_pass(0, 384, "wcb1a", w2dma=True)
                            transp(3)

                        # ---- FFN2 (i-outer, resident w2) + FFN1-B + LN2 ----
                        with contextlib.ExitStack() as f2_stack:
                            ln2p = f2_stack.enter_context(
                                tc.tile_pool(name="ln2p", bufs=1))
                            psO = f2_stack.enter_context(
                                tc.tile_pool(name="psO", bufs=2, space="PSUM"))

                            def emit_ln2(i, fo):
                                stat = ln2p.tile([128, 4], F32, name="st2",
                                                 tag="st2", bufs=2)
                                cent = ln2p.tile([128, D], F32, name="ce2",
                                                 tag="ce2")
                                sq = ln2p.tile([128, D], BF16, name="sq2",
                                               tag="sq2")
                                nc.vector.tensor_reduce(stat[:, 0:1], fo[:],
                                                        mybir.AxisListType.X,
                                                        OP.add)
                                nc.vector.tensor_scalar_mul(stat[:, 1:2],
                                                            stat[:, 0:1],
                                                            1.0 / D)
                                nc.vector.tensor_scalar_sub(cent[:], fo[:],
                                                            stat[:, 1:2])
                                nc.vector.scalar_tensor_tensor(
                                    sq[:], fo[:], stat[:, 1:2], cent[:],
                                    op0=OP.subtract, op1=OP.mult,
                                    accum_out=stat[:, 2:3])
                                nc.scalar.activation(stat[:, 3:4], stat[:, 2:3],
                                                     AF.Ln, bias=epsb[:, 0:1],
                                                     scale=1.0 / D)
                                nc.scalar.activation(stat[:, 0:1], stat[:, 3:4],
                                                     AF.Exp, scale=-0.5)
                                t1 = ln2p.tile([128, D], F32, name="t1b",
                                               tag="t1b")
                                nc.vector.scalar_tensor_tensor(
                                    t1[:], cent[:], stat[:, 0:1], gb2_sb[:],
                                    op0=OP.mult, op1=OP.mult)
                                xo = ln2p.tile([128, D], F32, name="xo",
                                               tag="xo", bufs=2)
                                nc.vector.tensor_add(xo[:], t1[:], x1[i][:])
                                nc.vector.tensor_add(xo[:], xo[:], bb2_sb[:])
                                nc.sync.dma_start(
                                    out=out[i * 128:(i + 1) * 128, :],
                                    in_=xo[:])

                            def ffn2_row(i):
                                ps = psO.tile([128, D], F32, name="pso",
                                              tag="pso")
                                for t in range(NFT):
                                    for n2 in range(2):
                                        nc.tensor.matmul(
                                            ps[:, n2 * 512:(n2 + 1) * 512],
                                            ht[t][:, i * 128:(i + 1) * 128],
                                            w2row[t][:, n2 * 512:(n2 + 1) * 512],
                                            start=(t == 0), stop=(t == NFT - 1))
                                fo = ln2p.tile([128, D], F32, name="fo",
                                               tag="fo", bufs=2)
                                nc.vector.tensor_add(fo[:], ps[:], b2b_sb[:])
                                emit_ln2(i, fo)

                            for i in range(3):
                                ffn2_row(i)
                            ffn1_pass(384, 512, "wcb1b")
                            ffn2_row(3)

    nc.finalize()
    return nc


_NC = None


def _get_nc():
    global _NC
    if _NC is None:
        _NC = _build()
    return _NC


def _host_prep(batch_X, padding_mask, W_Q, W_K, W_V, W_O, W1, b1, W2, b2,
               gamma1, beta1, gamma2, beta2):
    import ml_dtypes
    f = np.float32
    bf = ml_dtypes.bfloat16
    X = np.asarray(batch_X, f)
    pm = np.asarray(padding_mask)

    def colblocks(w, nt, dt=f):
        nd = w.shape[0] // 128
        return np.ascontiguousarray(
            np.asarray(w, f).reshape(nd, 128, nt, 128).transpose(2, 1, 0, 3)
            .astype(dt)).reshape(nt, 128, w.shape[0])

    shared = {
        "w1b": colblocks(np.asarray(W1, f), NFT, bf),
        "w2": np.ascontiguousarray(np.asarray(W2, f).astype(bf)),
        "b1m": np.ascontiguousarray(np.asarray(b1, f).reshape(NFT, 128).T),
        "gb1": np.ascontiguousarray(
            np.broadcast_to(np.asarray(gamma1, f), (128, D)).astype(bf)),
        "gb2": np.ascontiguousarray(
            np.broadcast_to(np.asarray(gamma2, f), (128, D)).astype(bf)),
        "bb2": np.ascontiguousarray(
            np.broadcast_to(np.asarray(beta2, f), (128, D))),
        "b2b": np.ascontiguousarray(
            np.broadcast_to(np.asarray(b2, f), (128, D))),
    }
    WQ, WK, WV, WO = (np.asarray(w, f) for w in (W_Q, W_K, W_V, W_O))
    be1 = np.asarray(beta1, f)
    in_maps = []
    for core in range(NCORES):
        b = core // 4
        r = core % 4
        hs = slice(r * 256, (r + 1) * 256)
        rows = np.concatenate(
            [np.arange(j * 512 + r * 128, j * 512 + r * 128 + 128)
             for j in range(NQ)])
        m = dict(shared)
        m["xt"] = np.ascontiguousarray(X[b].T.astype(bf))
        m["wqh"] = colblocks(WQ[:, hs], 2, bf)
        m["wkh"] = colblocks(WK[:, hs], 2, bf)
        m["wvh"] = np.ascontiguousarray(WV[:, hs].astype(bf))
        m["woh"] = np.ascontiguousarray(
            WO[hs, :].reshape(2, 128, D).astype(bf))
        m["keep"] = np.ascontiguousarray(
            (pm[b][rows] != 0).astype(f).reshape(NQ, 128).T)
        m["xqb"] = np.ascontiguousarray(X[b][rows] + be1)
        in_maps.append(m)
    return in_maps


def kernel(**inputs):
    nc = _get_nc()
    in_maps = _host_prep(**inputs)
    res = bass_utils.run_bass_kernel_spmd(nc, in_maps, list(range(NCORES)))
    out = np.empty((B, S, D), np.float32)
    for core in range(NCORES):
        b = core // 4
        r = core % 4
        for j in range(NQ):
            out[b, j * 512 + r * 128:j * 512 + r * 128 + 128] = \
                res.results[core]["out"][j * 128:(j + 1) * 128]
    return out


# revision 39
# speedup vs baseline: 1.1936x; 1.0088x over previous
"""Trainium2 Bass kernel for a dense transformer encoder layer — v3.

Sharding: tensor-parallel attention within each batch group of 4 cores.
Core (b=core//4, r=core%4) computes Q/K/V and attention for heads
4r..4r+3 over ALL 2048 rows of batch b, multiplies by its 256-row slice
of W_O (per-core weight content), and a pipelined 4-core ReduceScatter
(bf16) per q-quarter sums the partial attention outputs, delivering
each core its own row-tiles rank-independently.  LN1/residual, FFN and
LN2 then run row-parallel.

v3 changes vs v2.1:
- scores PSUM rotation deepened to 3 tiles (6 banks): breaks the
  exp -> bank-reuse -> scores dependency cycle that held the PE at a
  ~2us/step period and kept the HAM clock gate at K=4/8 (1.2 GHz).
- W_O partials and the 1/den broadcast matmuls borrow slots from the
  scores rotation (tag "sps") instead of a dedicated psW pool, freeing
  the 2 banks the deeper scores rotation needs.
- W_O emitted as 2 accumulating matmuls of N=1024 per row-tile
  (moving operand bf16 allows 1024 free dim) instead of 4 of N=512:
  halves the serialized LDWEIGHTS count.
- Q/K projections reordered (weights stationary per (t2,qk,c), two
  N=1024 matmuls per load over the 4 quarters) - 64 instead of 128
  matmuls.
- FFN2: w2 fully resident in SBUF (DMAed on the gpsimd queue during
  attention), i-outer loop with one [128,1024] psum per row-tile and
  32 accumulating N=1024 matmuls; LN2(i) and the output DMA overlap
  the next row-tile's matmul stream.  FFN2-A (row-tiles 0-2) runs
  before FFN1 pass B so only FFN2-B depends on the last-quarter ht.
"""

import contextlib

import numpy as np

import concourse.bass as bass
import concourse.mybir as mybir
from concourse import bacc
from concourse import bass_utils
from concourse.masks import make_identity
from concourse.tile import TileContext

F32 = mybir.dt.float32
F32R = mybir.dt.float32r
BF16 = mybir.dt.bfloat16
AF = mybir.ActivationFunctionType
OP = mybir.AluOpType

B, S, D, H, DKH, DFF = 2, 2048, 1024, 16, 64, 4096
EPS = 1e-6
NCORES = 8
GROUPS = [[0, 1, 2, 3], [4, 5, 6, 7]]
NDC = D // 128          # 8 contraction chunks
NKT = S // 128          # 16 key tiles
NQ = S // 512           # 4 q-quarters
NFT = DFF // 128        # 32 FFN tiles
VP = DKH + 1            # 65
LN_EXP_TABLE = 6        # natural_log_exp_and_others in act_info.json


def _build():
    nc = bacc.Bacc(None, num_devices=NCORES)

    xt = nc.declare_dram_parameter("xt", [D, S], BF16, isOutput=False)
    wqh = nc.declare_dram_parameter("wqh", [2, 128, D], BF16, isOutput=False)
    wkh = nc.declare_dram_parameter("wkh", [2, 128, D], BF16, isOutput=False)
    wvh = nc.declare_dram_parameter("wvh", [D, 256], BF16, isOutput=False)
    woh = nc.declare_dram_parameter("woh", [2, 128, D], BF16, isOutput=False)
    w1b = nc.declare_dram_parameter("w1b", [NFT, 128, D], BF16, isOutput=False)
    w2 = nc.declare_dram_parameter("w2", [DFF, D], BF16, isOutput=False)
    b1m = nc.declare_dram_parameter("b1m", [128, NFT], F32, isOutput=False)
    gb1 = nc.declare_dram_parameter("gb1", [128, D], BF16, isOutput=False)
    gb2 = nc.declare_dram_parameter("gb2", [128, D], BF16, isOutput=False)
    bb2 = nc.declare_dram_parameter("bb2", [128, D], F32, isOutput=False)
    b2b = nc.declare_dram_parameter("b2b", [128, D], F32, isOutput=False)
    keep = nc.declare_dram_parameter("keep", [128, NQ], F32, isOutput=False)
    xqb = nc.declare_dram_parameter("xqb", [512, D], F32, isOutput=False)
    out = nc.declare_dram_parameter("out", [512, D], F32, isOutput=True)

    with TileContext(nc) as tc:
        with tc.tile_pool(name="constp", bufs=1) as constp, \
             tc.tile_pool(name="dramp", bufs=1, space="DRAM") as dramp, \
             tc.tile_pool(name="persist", bufs=1) as persist:
            # pin the act table once: exp/ln/relu/copy all live in it
            nc.scalar.add_instruction(mybir.InstLoadActFuncSet(
                name=f"I-{nc.next_id()}", ins=[], outs=[],
                act_func_set_id=LN_EXP_TABLE))
            epsb = constp.tile([128, 1], F32, name="epsb")
            nc.vector.memset(epsb[:], EPS)
            ones_f = constp.tile([128, 128], F32, name="ones_f")
            nc.vector.memset(ones_f[:], 1.0)
            ones64 = constp.tile([128, 128], F32R, name="ones64")
            nc.vector.tensor_copy(ones64[:], ones_f[:])
            identity = constp.tile([128, 128], F32, name="identity")
            make_identity(nc, identity[:])

            # warmup collective: full 1MB like the real per-quarter RS so the
            # first production RS doesn't pay channel-rampup; staged on the
            # gpsimd queue to keep the sync queue free for the xt chunks.
            wz = constp.tile([128, D], BF16, name="wz")
            nc.vector.memset(wz[:], 0.0)
            biw = dramp.tile([4, 128, D], BF16, name="biw")
            bow = dramp.tile([128, D], BF16, name="bow")
            for g in range(4):
                nc.gpsimd.dma_start(out=biw[g], in_=wz[:])
            nc.gpsimd.collective_compute(
                "ReduceScatter", OP.add, replica_groups=GROUPS,
                ins=[biw[:].opt()], outs=[bow[:].opt()])

            bi = [dramp.tile([4, 128, D], BF16, name=f"bi{j}") for j in range(NQ)]
            bo = [dramp.tile([128, D], BF16, name=f"bo{j}") for j in range(NQ)]

            x1 = [persist.tile([128, D], F32, name=f"x1_{i}", tag=f"x1_{i}")
                  for i in range(NQ)]

            with tc.tile_pool(name="lncp", bufs=1) as lncp, \
                 tc.tile_pool(name="lnp", bufs=1) as lnp:
                attns_st = contextlib.ExitStack()
                attns = attns_st.enter_context(tc.tile_pool(name="attns", bufs=1))
                expp = attns_st.enter_context(tc.tile_pool(name="expp", bufs=4))
                wostg = attns_st.enter_context(tc.tile_pool(name="wostg", bufs=2))
                q_sb = [attns.tile([128, S], BF16, name=f"q{t}", tag=f"q{t}")
                        for t in range(2)]
                k_sb = [attns.tile([128, S], BF16, name=f"k{t}", tag=f"k{t}")
                        for t in range(2)]
                v_sb = [attns.tile([128, 4 * VP], BF16, name=f"v{t}", tag=f"v{t}")
                        for t in range(NKT)]
                ctxa = [attns.tile([128, S], BF16, name=f"ca{t}", tag=f"ca{t}")
                        for t in range(2)]
                wo_sb = [attns.tile([128, D], BF16, name=f"wo{t}", tag=f"wo{t}")
                         for t in range(2)]

                # ---- projections ----
                with tc.tile_pool(name="xthp", bufs=1) as xthp, \
                     tc.tile_pool(name="wproj", bufs=1) as wproj, \
                     tc.tile_pool(name="psP", bufs=8, space="PSUM") as psP:
                    wq_sb = [wproj.tile([128, D], BF16, name=f"wq{t}", tag=f"wq{t}")
                             for t in range(2)]
                    wk_sb = [wproj.tile([128, D], BF16, name=f"wk{t}", tag=f"wk{t}")
                             for t in range(2)]
                    wvc = [wproj.tile([128, 256], BF16, name=f"wv{c}", tag=f"wv{c}")
                           for c in range(NDC)]
                    xth = [xthp.tile([128, S], BF16, name=f"xth{c}", tag=f"xth{c}")
                           for c in range(NDC)]
                    # t2=0 weights first, then xt chunks split across two DMA
                    # queues, so the c-outer matmul groups below chase the
                    # arriving chunks instead of waiting for all of xt.
                    nc.sync.dma_start(out=wq_sb[0][:], in_=wqh[0])
                    nc.scalar.dma_start(out=wk_sb[0][:], in_=wkh[0])
                    for c in range(NDC):
                        eng = nc.sync if c % 2 == 0 else nc.scalar
                        eng.dma_start(out=xth[c][:],
                                      in_=xt[c * 128:(c + 1) * 128, :])
                    nc.sync.dma_start(out=wq_sb[1][:], in_=wqh[1])
                    nc.scalar.dma_start(out=wk_sb[1][:], in_=wkh[1])
                    for c in range(NDC):
                        eng = nc.sync if c % 2 == 0 else nc.scalar
                        eng.dma_start(out=wvc[c][:],
                                      in_=wvh[c * 128:(c + 1) * 128, :])
                    for t2 in range(2):
                        nc.sync.dma_start(out=wo_sb[t2][:], in_=woh[t2])

                    # Q/K: c-outer over an 8-psum group per t2 so the first
                    # matmuls start as soon as chunk 0 lands.
                    for t2 in range(2):
                        pss = {}
                        for n in range(NQ):
                            for qk in range(2):
                                pss[(n, qk)] = psP.tile([128, 512], F32,
                                                        name="psp", tag="psp")
                        for c in range(NDC):
                            for n in range(NQ):
                                for qk, w_sb in ((0, wq_sb), (1, wk_sb)):
                                    nc.tensor.matmul(
                                        pss[(n, qk)][:],
                                        w_sb[t2][:, c * 128:(c + 1) * 128],
                                        xth[c][:, n * 512:(n + 1) * 512],
                                        start=(c == 0), stop=(c == NDC - 1))
                        for n in range(NQ):
                            for qk, dst in ((0, q_sb), (1, k_sb)):
                                nc.vector.tensor_copy(
                                    dst[t2][:, n * 512:(n + 1) * 512],
                                    pss[(n, qk)][:])

                    for kt in range(NKT):
                        vr = v_sb[kt][:].rearrange("p (h c) -> p h c", c=VP)
                        nc.vector.tensor_copy(vr[:, :, DKH], ones_f[:, 0:4])
                    for kt in range(NKT):
                        ps = psP.tile([128, 512], F32, name="psv", tag="psp")
                        for c in range(NDC):
                            nc.tensor.matmul(
                                ps[:, 0:256], xth[c][:, kt * 128:(kt + 1) * 128],
                                wvc[c][:],
                                start=(c == 0), stop=(c == NDC - 1))
                        vr = v_sb[kt][:].rearrange("p (h c) -> p h c", c=VP)
                        nc.vector.tensor_copy(
                            vr[:, :, 0:DKH],
                            ps[:, 0:256].rearrange("p (h c) -> p h c", c=DKH))

                # ---- attention + lazy W_O + pipelined ReduceScatter ----
                if True:
                    att_ps = contextlib.ExitStack()
                    psS = att_ps.enter_context(
                        tc.tile_pool(name="psS", bufs=3, space="PSUM"))
                    psC = att_ps.enter_context(
                        tc.tile_pool(name="psC", bufs=2, space="PSUM"))
                    keep_sb = lncp.tile([128, NQ], F32, name="keep_sb")
                    nc.sync.dma_start(out=keep_sb[:], in_=keep[:, :])
                    gb1_sb = lncp.tile([128, D], BF16, name="gb1_sb")
                    nc.sync.dma_start(out=gb1_sb[:], in_=gb1[:, :])
                    xqb_t = {}

                    def emit_norm(p):
                        t2_, h2_, n_, rcp = p
                        rb = psS.tile([128, 1024], F32, name="psn", tag="sps")
                        nc.tensor.matmul(rb[:, 0:512], ones64[0:1, :],
                                         rcp[0:1, :], start=True, stop=True)
                        rbs = expp.tile([128, 512], BF16, name="rbs", tag="rbs",
                                        bufs=2)
                        nc.vector.tensor_copy(
                            rbs[h2_ * 64:(h2_ + 1) * 64, :],
                            rb[h2_ * 64:(h2_ + 1) * 64, 0:512])
                        sl = ctxa[t2_][h2_ * 64:(h2_ + 1) * 64,
                                       n_ * 512:(n_ + 1) * 512]
                        nc.vector.tensor_mul(sl, sl,
                                             rbs[h2_ * 64:(h2_ + 1) * 64, :])

                    def emit_wo_p(n, p):
                        qi = 4 * n + p
                        ps = psS.tile([128, 1024], F32, name="psw",
                                      tag="sps")
                        for n2 in range(2):
                            for t2_ in range(2):
                                nc.tensor.matmul(
                                    ps[:, n2 * 512:(n2 + 1) * 512],
                                    ctxa[t2_][:, qi * 128:(qi + 1) * 128],
                                    wo_sb[t2_][:, n2 * 512:(n2 + 1) * 512],
                                    start=(t2_ == 0), stop=(t2_ == 1))
                        stg = wostg.tile([128, D], BF16, name="stg",
                                         tag="stg")
                        nc.vector.tensor_copy(stg[:], ps[:])
                        nc.sync.dma_start(out=bi[n][p], in_=stg[:])

                    ao_t = {}

                    def emit_ao_dma(j, eng=None):
                        # bo[j] -> SBUF pull for LN1(j).  Placed on the gpsimd
                        # queue (idle between RS triggers) because the issuing
                        # engine blocks until RS_j completes - on the sync
                        # queue that head-of-line-blocks the bi staging DMAs.
                        ao = lnp.tile([128, D], BF16, name="ao", tag="ao",
                                      bufs=2)
                        (eng or nc.gpsimd).dma_start(out=ao[:], in_=bo[j][:])
                        ao_t[j] = ao

                    def emit_rs(n):
                        nc.gpsimd.collective_compute(
                            "ReduceScatter", OP.add, replica_groups=GROUPS,
                            ins=[bi[n][:].opt()], outs=[bo[n][:].opt()])
                        if n >= 1:
                            emit_ao_dma(n - 1)
                        xq = lncp.tile([128, D], F32, name="xqb", tag="xqb",
                                       bufs=2)
                        nc.sync.dma_start(out=xq[:],
                                          in_=xqb[n * 128:(n + 1) * 128, :])
                        xqb_t[n] = xq

                    def emit_ln1(j):
                        ao = ao_t.pop(j)
                        nc.vector.tensor_scalar_mul(ao[:], ao[:],
                                                    keep_sb[:, j:j + 1])
                        stat = lnp.tile([128, 4], F32, name="stat", tag="stat")
                        cent = lnp.tile([128, D], BF16, name="cent", tag="cent")
                        sq = lnp.tile([128, D], BF16, name="sq", tag="sq")
                        nc.vector.tensor_reduce(stat[:, 0:1], ao[:],
                                                mybir.AxisListType.X, OP.add)
                        nc.vector.tensor_scalar_mul(stat[:, 1:2], stat[:, 0:1],
                                                    1.0 / D)
                        nc.vector.tensor_scalar_sub(cent[:], ao[:],
                                                    stat[:, 1:2])
                        nc.vector.scalar_tensor_tensor(
                            sq[:], ao[:], stat[:, 1:2], cent[:],
                            op0=OP.subtract, op1=OP.mult, accum_out=stat[:, 2:3])
                        nc.scalar.activation(stat[:, 3:4], stat[:, 2:3], AF.Ln,
                                             bias=epsb[:, 0:1], scale=1.0 / D)
                        nc.scalar.activation(stat[:, 0:1], stat[:, 3:4], AF.Exp,
                                             scale=-0.5)
                        t1 = lnp.tile([128, D], F32, name="t1", tag="t1")
                        nc.vector.scalar_tensor_tensor(
                            t1[:], cent[:], stat[:, 0:1], gb1_sb[:],
                            op0=OP.mult, op1=OP.mult)
                        nc.vector.tensor_add(x1[j][:], t1[:],
                                             xqb_t.pop(j)[:])

                    norm_q = []
                    pending_wo = None   # quarter whose W_O is not yet emitted
                    pending_ln = []     # quarters whose LN1 is not yet emitted
                    for n in range(NQ):
                        for t2 in range(2):
                            cps = {h2: psC.tile([VP, 512], F32, name="cps",
                                                tag="cps") for h2 in (0, 1)}
                            prev = None

                            def ctx_block(pk, pexs, stop):
                                for h2 in (0, 1):
                                    h = 2 * t2 + h2
                                    for u in (0, 1):
                                        kt = 2 * pk + u
                                        nc.tensor.matmul(
                                            cps[h2][:],
                                            v_sb[kt][:, h * VP:(h + 1) * VP],
                                            pexs[h2][:, u * 512:(u + 1) * 512],
                                            start=(pk == 0 and u == 0),
                                            stop=(stop and u == 1))

                            for kt2 in range(NKT // 2):
                                exs = {}
                                for h2 in (0, 1):
                                    sps = psS.tile([128, 1024], F32, name="sps",
                                                   tag="sps")
                                    for u in (0, 1):
                                        kt = 2 * kt2 + u
                                        nc.tensor.matmul(
                                            sps[:, u * 512:(u + 1) * 512],
                                            k_sb[t2][h2 * 64:(h2 + 1) * 64,
                                                     kt * 128:(kt + 1) * 128],
                                            q_sb[t2][h2 * 64:(h2 + 1) * 64,
                                                     n * 512:(n + 1) * 512],
                                            start=True, stop=True)
                                    ex = expp.tile([128, 1024], BF16, name="ex",
                                                   tag="ex")
                                    nc.scalar.activation(ex[:], sps[:], AF.Exp,
                                                         scale=0.125)
                                    exs[h2] = ex
                                if prev is not None:
                                    ctx_block(prev[0], prev[1], stop=False)
                                prev = (kt2, exs)
                                # boundary work, spread through the block so
                                # the scores/exp pipeline never sees a burst:
                                # the previous quarter's norms, W_O tiles and
                                # RS trigger ride in one-per-iteration.
                                if kt2 == 2:
                                    while norm_q:
                                        emit_norm(norm_q.pop(0))
                                if t2 == 0 and pending_wo is not None:
                                    if 3 <= kt2 <= 6:
                                        emit_wo_p(pending_wo, kt2 - 3)
                                    if kt2 == 6:
                                        emit_rs(pending_wo)
                                        pending_ln.append(pending_wo)
                                        pending_wo = None
                                if t2 == 1 and kt2 == 5 and len(pending_ln) > 1:
                                    emit_ln1(pending_ln.pop(0))
                            ctx_block(prev[0], prev[1], stop=True)

                            # eager psum drain (ctx + den); 1/den on DVE for
                            # the t2=0 block (lag hides it), on the scalar
                            # engine (exp(-ln(den)), same table) for the t2=1
                            # block where ACT idles across the quarter
                            # boundary while DVE drains.
                            for h2 in (0, 1):
                                nc.vector.tensor_copy(
                                    ctxa[t2][h2 * 64:(h2 + 1) * 64,
                                             n * 512:(n + 1) * 512],
                                    cps[h2][0:DKH, :])
                            for h2 in (0, 1):
                                den_t = expp.tile([128, 512], F32, name="den",
                                                  tag="den", bufs=2)
                                rcp = expp.tile([128, 512], F32R, name="rcp",
                                                tag="rcp", bufs=3)
                                nc.vector.tensor_copy(den_t[0:1, :],
                                                      cps[h2][DKH:VP, :])
                                if t2 == 1:
                                    dln = expp.tile([128, 512], F32,
                                                    name="den", tag="den",
                                                    bufs=2)
                                    nc.scalar.activation(dln[0:1, :],
                                                         den_t[0:1, :], AF.Ln)
                                    with nc.allow_low_precision(
                                            reason="softmax 1/denom, f32r"):
                                        nc.scalar.activation(
                                            rcp[0:1, :], dln[0:1, :], AF.Exp,
                                            scale=-1.0)
                                else:
                                    with nc.allow_low_precision(
                                            reason="softmax 1/denom, f32r"):
                                        nc.vector.reciprocal(rcp[0:1, :],
                                                             den_t[0:1, :])
                                norm_q.append((t2, h2, n, rcp))
                        pending_wo = n

                    while norm_q:
                        emit_norm(norm_q.pop(0))
                    for p in range(4):
                        emit_wo_p(pending_wo, p)
                    emit_rs(pending_wo)   # also issues ao DMA for quarter 2
                    pending_ln.append(pending_wo)
                    emit_ln1(pending_ln.pop(0))   # LN1(2): RS_2 long done
                    att_ps.close()
                    attns_st.close()

                    # ---- tail: overlap last RS with transposes + FFN1-A ----
                    with contextlib.ExitStack() as tail_stack:
                        # w2 resident for FFN2: DMAed one row-block per
                        # FFN1-A iteration on the gpsimd queue (a single
                        # up-front burst of 8MB stalls the gpsimd engine for
                        # ~45us and everything queued behind it).
                        w2p = tail_stack.enter_context(
                            tc.tile_pool(name="w2p", bufs=1))
                        w2row = [w2p.tile([128, D], BF16, name=f"w2r{t}",
                                          tag=f"w2r{t}") for t in range(NFT)]
                        x1tp = tail_stack.enter_context(
                            tc.tile_pool(name="x1tp", bufs=1))
                        hp = tail_stack.enter_context(
                            tc.tile_pool(name="hp", bufs=1))
                        wstr4 = tail_stack.enter_context(
                            tc.tile_pool(name="wstr4", bufs=4))
                        bp = tail_stack.enter_context(
                            tc.tile_pool(name="bp", bufs=1))
                        ln2c = tail_stack.enter_context(
                            tc.tile_pool(name="ln2c", bufs=1))
                        psF = tail_stack.enter_context(
                            tc.tile_pool(name="psF", bufs=3, space="PSUM"))
                        x1t = [x1tp.tile([128, 512], BF16, name=f"x1t{c}",
                                         tag=f"x1t{c}")
                               for c in range(NDC)]
                        b1_sb = bp.tile([128, NFT], F32, name="b1_sb")
                        nc.sync.dma_start(out=b1_sb[:], in_=b1m[:, :])
                        gb2_sb = ln2c.tile([128, D], BF16, name="gb2_sb")
                        nc.sync.dma_start(out=gb2_sb[:], in_=gb2[:, :])
                        bb2_sb = ln2c.tile([128, D], F32, name="bb2_sb")
                        nc.sync.dma_start(out=bb2_sb[:], in_=bb2[:, :])
                        b2b_sb = ln2c.tile([128, D], F32, name="b2b_sb")
                        nc.sync.dma_start(out=b2b_sb[:], in_=b2b[:, :])
                        ht = [hp.tile([128, 512], BF16, name=f"ht{t}",
                                      tag=f"ht{t}")
                              for t in range(NFT)]

                        def ffn1_pass(lo, hi, tag, w2dma=False):
                            for t in range(NFT):
                                if w2dma:
                                    nc.gpsimd.dma_start(
                                        out=w2row[t][:],
                                        in_=w2[t * 128:(t + 1) * 128, :])
                                    if t == 16:
                                        # RS_3 should be done by now; pull
                                        # bo[3] for the post-pass LN1(3)
                                        emit_ao_dma(3, eng=nc.sync)
                                wcb = wstr4.tile([128, D], BF16, name="wcb1",
                                                 tag=tag)
                                nc.sync.dma_start(out=wcb[:], in_=w1b[t])
                                ps = psF.tile([128, 512], F32, name="psh",
                                              tag="psh")
                                for c in range(NDC):
                                    nc.tensor.matmul(
                                        ps[:, lo:hi],
                                        wcb[:, c * 128:(c + 1) * 128],
                                        x1t[c][:, lo:hi],
                                        start=(c == 0), stop=(c == NDC - 1))
                                nc.scalar.activation(
                                    ht[t][:, lo:hi], ps[:, lo:hi], AF.Relu,
                                    bias=b1_sb[:, t:t + 1])

                        with contextlib.ExitStack() as t_stack:
                            psT = t_stack.enter_context(
                                tc.tile_pool(name="psT", bufs=3, space="PSUM"))

                            def transp(i):
                                for c in range(NDC):
                                    ps = psT.tile([128, 128], F32, name="pst",
                                                  tag="pst")
                                    nc.tensor.transpose(
                                        ps[:], x1[i][:, c * 128:(c + 1) * 128],
                                        identity[:])
                                    nc.vector.tensor_copy(
                                        x1t[c][:, i * 128:(i + 1) * 128], ps[:])

                            # i-outer: quarters 0/1 transpose immediately;
                            # quarter 2 trails its LN1 chain
                            for i in range(3):
                                transp(i)
                            # covers RS_3 + LN1(3); also streams in w2 and
                            # (at t==16) the bo[3] pull
                            ffn1_pass(0, 384, "wcb1a", w2dma=True)
                            emit_ln1(pending_ln.pop(0))  # LN1(3)
                            transp(3)

                        # ---- FFN2 (i-outer, resident w2) + FFN1-B + LN2 ----
                        with contextlib.ExitStack() as f2_stack:
                            ln2p = f2_stack.enter_context(
                                tc.tile_pool(name="ln2p", bufs=1))
                            psO = f2_stack.enter_context(
                                tc.tile_pool(name="psO", bufs=2, space="PSUM"))

                            def emit_ln2(i, fo):
                                stat = ln2p.tile([128, 4], F32, name="st2",
                                                 tag="st2", bufs=2)
                                cent = ln2p.tile([128, D], F32, name="ce2",
                                                 tag="ce2")
                                sq = ln2p.tile([128, D], BF16, name="sq2",
                                               tag="sq2")
                                nc.vector.tensor_reduce(stat[:, 0:1], fo[:],
                                                        mybir.AxisListType.X,
                                                        OP.add)
                                nc.vector.tensor_scalar_mul(stat[:, 1:2],
                                                            stat[:, 0:1],
                                                            1.0 / D)
                                nc.vector.tensor_scalar_sub(cent[:], fo[:],
                                                            stat[:, 1:2])
                                nc.vector.scalar_tensor_tensor(
                                    sq[:], fo[:], stat[:, 1:2], cent[:],
                                    op0=OP.subtract, op1=OP.mult,
                                    accum_out=stat[:, 2:3])
                                nc.scalar.activation(stat[:, 3:4], stat[:, 2:3],
                                                     AF.Ln, bias=epsb[:, 0:1],
                                                     scale=1.0 / D)
                                nc.scalar.activation(stat[:, 0:1], stat[:, 3:4],
                                                     AF.Exp, scale=-0.5)
                                t1 = ln2p.tile([128, D], F32, name="t1b",
                                               tag="t1b")
                                nc.vector.scalar_tensor_tensor(
                                    t1[:], cent[:], stat[:, 0:1], gb2_sb[:],
                                    op0=OP.mult, op1=OP.mult)
                                xo = ln2p.tile([128, D], F32, name="xo",
                                               tag="xo", bufs=2)
                                nc.vector.tensor_add(xo[:], t1[:], x1[i][:])
                                nc.vector.tensor_add(xo[:], xo[:], bb2_sb[:])
                                nc.sync.dma_start(
                                    out=out[i * 128:(i + 1) * 128, :],
                                    in_=xo[:])

                            def ffn2_row(i):
                                ps = psO.tile([128, D], F32, name="pso",
                                              tag="pso")
                                for t in range(NFT):
                                    for n2 in range(2):
                                        nc.tensor.matmul(
                                            ps[:, n2 * 512:(n2 + 1) * 512],
                                            ht[t][:, i * 128:(i + 1) * 128],
                                            w2row[t][:, n2 * 512:(n2 + 1) * 512],
                                            start=(t == 0), stop=(t == NFT - 1))
                                fo = ln2p.tile([128, D], F32, name="fo",
                                               tag="fo", bufs=2)
                                nc.vector.tensor_add(fo[:], ps[:], b2b_sb[:])
                                emit_ln2(i, fo)

                            for i in range(3):
                                ffn2_row(i)
                            ffn1_pass(384, 512, "wcb1b")
                            ffn2_row(3)

    nc.finalize()
    return nc


_NC = None


def _get_nc():
    global _NC
    if _NC is None:
        _NC = _build()
    return _NC


def _host_prep(batch_X, padding_mask, W_Q, W_K, W_V, W_O, W1, b1, W2, b2,
               gamma1, beta1, gamma2, beta2):
    import ml_dtypes
    f = np.float32
    bf = ml_dtypes.bfloat16
    X = np.asarray(batch_X, f)
    pm = np.asarray(padding_mask)

    def colblocks(w, nt, dt=f):
        nd = w.shape[0] // 128
        return np.ascontiguousarray(
            np.asarray(w, f).reshape(nd, 128, nt, 128).transpose(2, 1, 0, 3)
            .astype(dt)).reshape(nt, 128, w.shape[0])

    shared = {
        "w1b": colblocks(np.asarray(W1, f), NFT, bf),
        "w2": np.ascontiguousarray(np.asarray(W2, f).astype(bf)),
        "b1m": np.ascontiguousarray(np.asarray(b1, f).reshape(NFT, 128).T),
        "gb1": np.ascontiguousarray(
            np.broadcast_to(np.asarray(gamma1, f), (128, D)).astype(bf)),
        "gb2": np.ascontiguousarray(
            np.broadcast_to(np.asarray(gamma2, f), (128, D)).astype(bf)),
        "bb2": np.ascontiguousarray(
            np.broadcast_to(np.asarray(beta2, f), (128, D))),
        "b2b": np.ascontiguousarray(
            np.broadcast_to(np.asarray(b2, f), (128, D))),
    }
    WQ, WK, WV, WO = (np.asarray(w, f) for w in (W_Q, W_K, W_V, W_O))
    be1 = np.asarray(beta1, f)
    in_maps = []
    for core in range(NCORES):
        b = core // 4
        r = core % 4
        hs = slice(r * 256, (r + 1) * 256)
        rows = np.concatenate(
            [np.arange(j * 512 + r * 128, j * 512 + r * 128 + 128)
             for j in range(NQ)])
        m = dict(shared)
        m["xt"] = np.ascontiguousarray(X[b].T.astype(bf))
        m["wqh"] = colblocks(WQ[:, hs], 2, bf)
        m["wkh"] = colblocks(WK[:, hs], 2, bf)
        m["wvh"] = np.ascontiguousarray(WV[:, hs].astype(bf))
        m["woh"] = np.ascontiguousarray(
            WO[hs, :].reshape(2, 128, D).astype(bf))
        m["keep"] = np.ascontiguousarray(
            (pm[b][rows] != 0).astype(f).reshape(NQ, 128).T)
        m["xqb"] = np.ascontiguousarray(X[b][rows] + be1)
        in_maps.append(m)
    return in_maps


def kernel(**inputs):
    nc = _get_nc()
    in_maps = _host_prep(**inputs)
    res = bass_utils.run_bass_kernel_spmd(nc, in_maps, list(range(NCORES)))
    out = np.empty((B, S, D), np.float32)
    for core in range(NCORES):
        b = core // 4
        r = core % 4
        for j in range(NQ):
            out[b, j * 512 + r * 128:j * 512 + r * 128 + 128] = \
                res.results[core]["out"][j * 128:(j + 1) * 128]
    return out
